# revision 28
# baseline (speedup 1.0000x reference)
"""Trainium2 Bass kernel for the Sinkhorn-OT loss problem.

Math summary (mirrors the reference):
  mapped = einsum('bf,bdf->bd', fea, W[y]) + bW[y];  a = softmax(mapped)
  b = softmax(anchor_tab[y]);  M_ij = (i-j)^2 on D=1024, eps=0.05
  ot(x, y) = 50-iteration log-domain Sinkhorn value
  losses = 2*ot(a,b) - ot(a,a) - ot(b,b);  weights = softmax(-losses)
  loss = sum(losses*weights)

Key structural facts exploited:
  * The Gibbs kernel exp(-(i-j)^2/eps) decays by e^-20 per unit distance, so
    every logsumexp over D is exactly banded with radius 3 in f32 (terms at
    |d|=4 sit >= 40 below the row max - invisible at f32 precision).
  * ot(b,b) has only 5 distinct instances (one per class).
  * The warm-start stabilizer (previous iteration's lse output) keeps all
    exp arguments in [-inf, ~7]; no per-row max pass is needed.

Distribution over the 8 cores:
  Phase A: D-parallel matmul. Core k holds W[:, 128k:128k+128, :], computes
    that D-slice of mapped for all 64 samples on the TensorEngine, then an
    AllToAll redistributes slices so core k owns full rows for samples
    8k..8k+8.
  Phase B: problem-parallel Sinkhorn. Each core runs 17 independent solves
    in two interleaved groups (A: 8 ab + 1 bb, B: 8 aa) so the two serial
    update chains fill each other's pipeline gaps. Each problem's D=1024
    axis is folded as [128 partitions x 8 cols] plus a 3-wide halo
    maintained by TensorEngine shift-matmuls.
  Host: assembles the 133 values, forms losses/weights/loss (O(64) work).

Problem order in the "values" output: [0:8] ab, [8] bb, [9:17] aa.
"""

import numpy as np

NUM_CLASSES = 5
FEAT = 2048
D = 1024
B = 64
EPS = 0.05
ITERS = 50
TEMP = 1.0

R = 3                 # band radius
HW = 8 + 2 * R        # halo'd fold width (14)
NCORES = 8
SAMP = B // NCORES    # samples per core (8)
NPROB = 2 * SAMP + 1  # problems per core (17)
DSL = D // NCORES     # D-slice per core (128)
FOLD = D // 128       # fold width (8)
NEG = -1.0e30
import os
EXP_SPLIT = int(os.environ.get('EXP_SPLIT', '1'))
HALO_DVE = int(os.environ.get('HALO_DVE', '0'))
NA = SAMP + 1         # group A: 8 ab + 1 bb
NB = SAMP             # group B: 8 aa


def _build_program(iters=ITERS, collective=True):
    import concourse.bass as bass
    import concourse.bacc as bacc
    import concourse.tile as tile
    from concourse import mybir

    f32 = mybir.dt.float32
    AF = mybir.ActivationFunctionType
    ALU = mybir.AluOpType
    AX = mybir.AxisListType

    nc = bacc.Bacc("TRN2", target_bir_lowering=False, debug=False)

    fea_d = nc.declare_dram_parameter("fea", [B, FEAT], f32, isOutput=False)
    wsl_d = nc.declare_dram_parameter("wsl", [NUM_CLASSES, DSL, FEAT], f32, isOutput=False)
    bwsl_d = nc.declare_dram_parameter("bwsl", [B, DSL], f32, isOutput=False)
    onehot_d = nc.declare_dram_parameter("onehot", [B, NUM_CLASSES], f32, isOutput=False)
    logbs_d = nc.declare_dram_parameter("logbs", [SAMP + 1, D], f32, isOutput=False)
    vals_d = nc.declare_dram_parameter("values", [NPROB], f32, isOutput=True)

    ident_d = nc.inline_tensor(np.eye(128, dtype=np.float32), name="ident")
    # shift-down: out[q] = in[q-1]  (SD[k, m] = 1 iff k = m-1)
    sd_d = nc.inline_tensor(np.eye(128, k=1, dtype=np.float32), name="sdn")
    # shift-up: out[q] = in[q+1]  (SU[k, m] = 1 iff k = m+1)
    su_d = nc.inline_tensor(np.eye(128, k=-1, dtype=np.float32), name="sup")
    ones_d = nc.inline_tensor(np.ones((1, 128), dtype=np.float32), name="ones1")
    bneg0_np = np.zeros((128, 1), dtype=np.float32); bneg0_np[0, 0] = NEG
    bneg127_np = np.zeros((128, 1), dtype=np.float32); bneg127_np[127, 0] = NEG
    bneg0_d = nc.inline_tensor(bneg0_np, name="bneg0")
    bneg127_d = nc.inline_tensor(bneg127_np, name="bneg127")
    # halo boundary mask: cols 0:3 NEG at partition 0, cols 3:6 NEG at partition 127
    hm_np = np.zeros((128, 6), dtype=np.float32)
    hm_np[0, 0:3] = NEG; hm_np[127, 3:6] = NEG
    hmask_d = nc.inline_tensor(hm_np, name="hmask")

    with tile.TileContext(nc) as tc:
        with (
            tc.tile_pool(name="consts", bufs=1) as consts,
            tc.tile_pool(name="pha", bufs=2) as pha,
            tc.tile_pool(name="state", bufs=1) as state,
            tc.tile_pool(name="scratch", bufs=2) as scratch,
            tc.tile_pool(name="psum_mm", bufs=2, space="PSUM") as psum_mm,
            tc.tile_pool(name="psum_tr", bufs=2, space="PSUM") as psum_tr,
            tc.tile_pool(name="psum_h", bufs=1, space="PSUM") as psum_h,
            tc.tile_pool(name="dram", bufs=1, space="DRAM") as dram,
        ):
            # ---------- constants to SBUF ----------
            ident = consts.tile([128, 128], f32, tag="ident")
            nc.sync.dma_start(ident, ident_d[:, :])
            sdn = consts.tile([128, 128], f32, tag="sdn")
            nc.sync.dma_start(sdn, sd_d[:, :])
            sup = consts.tile([128, 128], f32, tag="sup")
            nc.sync.dma_start(sup, su_d[:, :])
            ones1 = consts.tile([1, 128], f32, tag="ones1")
            nc.sync.dma_start(ones1, ones_d[:, :])
            onehot = consts.tile([B, NUM_CLASSES], f32, tag="onehot")
            nc.sync.dma_start(onehot, onehot_d[:, :])
            bwsl = consts.tile([B, DSL], f32, tag="bwsl")
            nc.sync.dma_start(bwsl, bwsl_d[:, :])
            bneg0 = consts.tile([128, 1], f32, tag="bneg0")
            nc.sync.dma_start(bneg0, bneg0_d[:, :])
            bneg127 = consts.tile([128, 1], f32, tag="bneg127")
            nc.sync.dma_start(bneg127, bneg127_d[:, :])
            hmask = consts.tile([128, 6], f32, tag="hmask")
            nc.sync.dma_start(hmask, hmask_d[:, :])

            # per-partition bias constants for activation (c_d = -20*d^2)
            cbias = {}
            for v in sorted({-20.0 * d * d for d in range(0, R + 1)}):
                t = consts.tile([128, 1], f32, tag=f"cb{int(-v)}")
                nc.vector.memset(t, v)
                cbias[v] = t

            # ---------- Phase A: mapped D-slice for all 64 samples ----------
            fea_sb = consts.tile([B, FEAT], f32, tag="fea")
            nc.sync.dma_start(fea_sb, fea_d[:, :])

            feaT = consts.tile([128, 16, B], f32, tag="feaT")
            for j in range(16):
                pt = psum_tr.tile([128, B], f32, tag="tr")
                nc.tensor.transpose(pt, fea_sb[:, j * 128:(j + 1) * 128], ident[0:B, 0:B])
                nc.scalar.copy(feaT[:, j, :], pt)

            mapped = state.tile([B, DSL], f32, tag="mapped")
            for c in range(NUM_CLASSES):
                wc = pha.tile([128, FEAT], f32, tag="wc")
                for h in range(4):
                    nc.sync.dma_start(wc[:, h * 512:(h + 1) * 512],
                                      wsl_d[c, :, h * 512:(h + 1) * 512])
                wcT = pha.tile([128, 16, DSL], f32, tag="wcT")
                for j in range(16):
                    ptw = psum_tr.tile([128, 128], f32, tag="tr")
                    nc.tensor.transpose(ptw, wc[:, j * 128:(j + 1) * 128], ident)
                    if j % 2 == 0:
                        nc.vector.tensor_copy(wcT[:, j, :], ptw)
                    else:
                        nc.scalar.copy(wcT[:, j, :], ptw)
                pmm = psum_mm.tile([B, DSL], f32, tag="pmm")
                for j in range(16):
                    nc.tensor.matmul(pmm, lhsT=feaT[:, j, :], rhs=wcT[:, j, :],
                                     start=(j == 0), stop=(j == 15))
                if c == 0:
                    nc.vector.tensor_scalar_mul(mapped, pmm, onehot[:, 0:1])
                else:
                    nc.vector.scalar_tensor_tensor(
                        out=mapped, in0=pmm, scalar=onehot[:, c:c + 1], in1=mapped,
                        op0=ALU.mult, op1=ALU.add)
            nc.vector.tensor_add(mapped, mapped, bwsl)

            # ---------- AllToAll: D-slices -> per-core full rows ----------
            ag_in = dram.tile([B, DSL], f32, tag="ag_in")
            ag_out = dram.tile([B, DSL], f32, tag="ag_out")
            nc.sync.dma_start(ag_in, mapped)
            if collective:
                nc.gpsimd.collective_compute(
                    "AllToAll", ALU.bypass,
                    replica_groups=[list(range(NCORES))],
                    ins=[ag_in[:, :].opt()], outs=[ag_out[:, :].opt()])
            else:
                # single-core timing variant: plain copy stands in for AllToAll
                nc.sync.dma_start(ag_out, ag_in[:, :])

            # ag_out[j*8 + p, :] = D-slice j of sample (8*core + p).
            # fold sample p's full row into [128, 8]: partition q holds D[8q..8q+8)
            mraw = state.tile([128, SAMP, FOLD], f32, tag="mraw")
            agv = ag_out[:, :].rearrange("(j p) (q c) -> j p q c", p=SAMP, c=FOLD)
            for p in range(SAMP):
                nc.sync.dma_start(mraw[:, p, :], agv[:, p, :, :])

            # ---------- loga = mraw - lse_D(mraw) per sample ----------
            def cross_partition_chain(src, nprob, op_alu):
                pr = scratch.tile([128, nprob], f32, tag="cp_pr")
                nc.vector.tensor_reduce(pr, src, axis=AX.X, op=op_alu)
                tp = psum_tr.tile([nprob, 128], f32, tag="tr")
                nc.tensor.transpose(tp, pr, ident)
                red = scratch.tile([nprob, 1], f32, tag="cp_red")
                nc.vector.tensor_reduce(red, tp, axis=AX.X, op=op_alu)
                return red

            def bcast_over_parts(col, nprob):
                tpc = psum_tr.tile([1, nprob], f32, tag="tr")
                nc.tensor.transpose(tpc, col, ident[0:nprob, 0:nprob])
                row = scratch.tile([1, nprob], f32, tag="bc_row")
                nc.scalar.copy(row, tpc)
                bc = psum_tr.tile([128, nprob], f32, tag="tr")
                nc.tensor.matmul(bc, lhsT=ones1, rhs=row, start=True, stop=True)
                return bc

            # mapped ~ N(0,1)-scale (|x| < ~10), so exp needs no max-stabilizer:
            # Za = ln(sum exp(mapped)) directly.
            e_e = scratch.tile([128, SAMP, FOLD], f32, tag="e_e")
            nc.scalar.activation(e_e, mraw, AF.Exp)
            ss_col = cross_partition_chain(e_e, SAMP, ALU.add)
            za = scratch.tile([SAMP, 1], f32, tag="za")
            nc.scalar.activation(za, ss_col, AF.Ln)
            bz = bcast_over_parts(za, SAMP)
            loga = state.tile([128, SAMP, FOLD], f32, tag="loga")
            nc.vector.tensor_sub(loga, mraw, bz[:, :, None].broadcast_to([128, SAMP, FOLD]))

            # ---------- marginals per group ----------
            # group A (9 probs): 0..7 ab (LA=loga_p, LB=logb_p), 8 bb (LA=LB=logb_cls)
            # group B (8 probs): aa (LA=LB=loga_p)
            NP = {0: NA, 1: NB}
            LA = {}; LB = {}
            LA[0] = state.tile([128, NA, FOLD], f32, tag="LA_A", name="LA_A")
            LB[0] = state.tile([128, NA, FOLD], f32, tag="LB_A", name="LB_A")
            LA[1] = state.tile([128, NB, FOLD], f32, tag="LA_B", name="LA_B")
            LB[1] = state.tile([128, NB, FOLD], f32, tag="LB_B", name="LB_B")
            nc.scalar.copy(LA[0][:, 0:SAMP, :], loga)
            nc.scalar.copy(LA[1][:, :, :], loga)
            nc.scalar.copy(LB[1][:, :, :], loga)
            for p in range(SAMP):
                nc.sync.dma_start(
                    LB[0][:, p, :],
                    logbs_d[p, :].rearrange("(q c) -> q c", c=FOLD))
            nc.sync.dma_start(
                LA[0][:, SAMP, :],
                logbs_d[SAMP, :].rearrange("(q c) -> q c", c=FOLD))
            nc.sync.dma_start(
                LB[0][:, SAMP, :],
                logbs_d[SAMP, :].rearrange("(q c) -> q c", c=FOLD))

            # ---------- Sinkhorn state (per group) ----------
            SFh = {}; SGh = {}; Mf = {}; Mg = {}; Tb = {}; Eb = {}
            acc = {}; lnacc = {}
            for g in (0, 1):
                s = "AB"[g]
                SFh[g] = state.tile([128, NP[g], HW], f32, tag=f"SFh{s}", name=f"SFh{s}")
                SGh[g] = state.tile([128, NP[g], HW], f32, tag=f"SGh{s}", name=f"SGh{s}")
                Mf[g] = state.tile([128, NP[g], FOLD], f32, tag=f"Mf{s}", name=f"Mf{s}")
                Mg[g] = state.tile([128, NP[g], FOLD], f32, tag=f"Mg{s}", name=f"Mg{s}")
                # band-slot order: [0]=d0, [1]=-1, [2]=+1, [3]=-2, [4]=+2, [5]=-3, [6]=+3
                Tb[g] = state.tile([128, NP[g], FOLD, 2 * R + 1], f32, tag=f"Tb{s}", name=f"Tb{s}")
                Eb[g] = state.tile([128, NP[g], FOLD, 2 * R + 1], f32, tag=f"Eb{s}", name=f"Eb{s}")
                acc[g] = state.tile([128, NP[g], FOLD], f32, tag=f"acc{s}", name=f"acc{s}")
                lnacc[g] = state.tile([128, NP[g], FOLD], f32, tag=f"lnacc{s}", name=f"lnacc{s}")
                nc.vector.memset(Mf[g], 0.0)
                nc.vector.memset(Mg[g], 0.0)
                nc.vector.memset(SFh[g], 0.0)
                nc.vector.memset(SGh[g], 0.0)

            def halo_fill(buf, g):
                np_ = NP[g]
                # both halos via one PSUM tile + one 2-range DVE add.
                # left halo: buf[q, :, 0:R] = buf[q-1, :, FOLD:FOLD+R]
                # right halo: buf[q, :, R+FOLD:HW] = buf[q+1, :, R:2R]
                # boundary partitions forced to NEG by the hmask tile.
                if HALO_DVE:
                    ph = psum_h.tile([128, 2, NA, R], f32, tag=f"halo{'AB'[g]}")
                    nc.tensor.matmul(ph[:, 0, 0:np_, :], lhsT=sdn,
                                     rhs=buf[:, :, FOLD:FOLD + R], start=True, stop=True)
                    nc.tensor.matmul(ph[:, 1, 0:np_, :], lhsT=sup,
                                     rhs=buf[:, :, R:2 * R], start=True, stop=True)
                    # one DVE add writes both halo col-ranges: dims (side, prob, col)
                    bb = buf[:, :, 0:R]
                    halo_out = bass.AP(tensor=bb.tensor, offset=bb.offset,
                                       ap=[bb.ap[0], [R + FOLD, 2], bb.ap[1], [1, R]])
                    ph_v = ph[:, :, 0:np_, :]
                    hs = hmask[:, :]
                    hm_v = bass.AP(tensor=hs.tensor, offset=hs.offset,
                                   ap=[hs.ap[0], [R, 2], [0, np_], [1, R]])
                    nc.vector.tensor_add(halo_out, ph_v, hm_v)
                else:
                    pl = psum_h.tile([128, NA, R], f32, tag=f"hl{'AB'[g]}")
                    nc.tensor.matmul(pl[:, 0:np_, :], lhsT=sdn,
                                     rhs=buf[:, :, FOLD:FOLD + R], start=True, stop=True)
                    nc.scalar.add(buf[:, :, 0:R], pl[:, 0:np_, :], bneg0[:, 0:1])
                    pr_ = psum_h.tile([128, NA, R], f32, tag=f"hr{'AB'[g]}")
                    nc.tensor.matmul(pr_[:, 0:np_, :], lhsT=sup,
                                     rhs=buf[:, :, R:2 * R], start=True, stop=True)
                    nc.scalar.add(buf[:, :, R + FOLD:HW], pr_[:, 0:np_, :],
                                  bneg127[:, 0:1])

            for g in (0, 1):
                nc.scalar.copy(SGh[g][:, :, R:R + FOLD], LB[g])
                nc.scalar.copy(SFh[g][:, :, R:R + FOLD], LA[g])
                halo_fill(SGh[g], g)

            def pair_ap(buf, k, np_):
                """[128, np, FOLD, 2] view of buf: pair (-k, +k) windows."""
                base = buf[:, :, R - k:R - k + FOLD]
                return bass.AP(tensor=base.tensor, offset=base.offset,
                               ap=[*base.ap, [2 * k, 2]])

            def bcast2_ap(t):
                """append a step-0 pair dim to a [128, np, FOLD] view."""
                return bass.AP(tensor=t.tensor, offset=t.offset,
                               ap=[*t.ap, [0, 2]])

            def half_update(g, src, dst, M, LAd):
                T, E = Tb[g], Eb[g]
                np_ = NP[g]
                # pair windows via 4D tensor_sub (walrus-legal); c_d goes into
                # the per-pair Exp bias (c identical for +-d).
                nc.vector.tensor_sub(T[:, :, :, 0], src[:, :, R:R + FOLD], M)
                for k in (1, 2, 3):
                    nc.vector.tensor_sub(
                        T[:, :, :, 2 * k - 1:2 * k + 1], pair_ap(src, k, np_),
                        bcast2_ap(M[:, :, :]))
                nc.scalar.activation(E[:, :, :, 0], T[:, :, :, 0], AF.Exp)
                for k in (1, 2, 3):
                    nc.scalar.activation(
                        E[:, :, :, 2 * k - 1:2 * k + 1], T[:, :, :, 2 * k - 1:2 * k + 1],
                        AF.Exp, bias=cbias[-20.0 * k * k][:, 0:1])
                nc.vector.tensor_reduce(acc[g], E, axis=AX.X, op=ALU.add)
                nc.scalar.activation(lnacc[g], acc[g], AF.Ln)
                # dst center: LA - M_new == dst_prev - lnacc (dst was LA - M_old);
                # keeps the halo shift off the M-update dependency
                nc.vector.tensor_sub(dst[:, :, R:R + FOLD],
                                     dst[:, :, R:R + FOLD], lnacc[g])
                nc.gpsimd.tensor_add(M, M, lnacc[g])
                halo_fill(dst, g)

            for _t in range(iters):
                half_update(0, SGh[0], SFh[0], Mf[0], LA[0])
                half_update(1, SGh[1], SFh[1], Mf[1], LA[1])
                half_update(0, SFh[0], SGh[0], Mg[0], LB[0])
                half_update(1, SFh[1], SGh[1], Mg[1], LB[1])

            # ---------- value: V = sum_{i,d} exp(SF_i+SG_{i+d}+c_d)(F_i+G_{i+d}) ----------
            vrows = {}
            for g in (0, 1):
                np_ = NP[g]
                Ft = scratch.tile([128, NA, FOLD], f32, tag="Ft", name="Ft")[:, 0:np_, :]
                nc.vector.tensor_sub(Ft, SFh[g][:, :, R:R + FOLD], LA[g])
                nc.vector.tensor_scalar_mul(Ft, Ft, float(EPS))
                Gh = scratch.tile([128, NA, HW], f32, tag="Gh", name="Gh")[:, 0:np_, :]
                nc.vector.tensor_sub(Gh[:, :, R:R + FOLD], SGh[g][:, :, R:R + FOLD], LB[g])
                nc.vector.tensor_scalar_mul(
                    Gh[:, :, R:R + FOLD], Gh[:, :, R:R + FOLD], float(EPS))
                halo_fill(Gh, g)

                s1 = scratch.tile([128, NA, FOLD], f32, tag="s1", name="s1")[:, 0:np_, :]
                pd_ = scratch.tile([128, NA, FOLD], f32, tag="pd", name="pd")[:, 0:np_, :]
                s2 = scratch.tile([128, NA, FOLD], f32, tag="s2", name="s2")[:, 0:np_, :]
                vacc = scratch.tile([128, NA, FOLD], f32, tag="vacc", name="vacc")[:, 0:np_, :]
                for di, d in enumerate(range(-R, R + 1)):
                    nc.vector.tensor_add(
                        s1, SFh[g][:, :, R:R + FOLD], SGh[g][:, :, R + d:R + d + FOLD])
                    nc.scalar.activation(pd_, s1, AF.Exp,
                                         bias=cbias[-20.0 * d * d][:, 0:1])
                    nc.vector.tensor_add(s2, Ft, Gh[:, :, R + d:R + d + FOLD])
                    if di == 0:
                        nc.vector.tensor_mul(vacc, pd_, s2)
                    else:
                        nc.vector.tensor_mul(s2, pd_, s2)
                        nc.vector.tensor_add(vacc, vacc, s2)

                vsum = scratch.tile([128, NA], f32, tag="vsum", name="vsum")[:, 0:np_]
                nc.vector.tensor_reduce(vsum, vacc, axis=AX.X, op=ALU.add)
                tv = psum_tr.tile([NA, 128], f32, tag="tr")
                nc.tensor.transpose(tv[0:np_, :], vsum, ident)
                vrow = scratch.tile([NA, 1], f32, tag=f"vrow{'AB'[g]}", name=f"vrow{'AB'[g]}")
                nc.vector.tensor_reduce(vrow[0:np_, :], tv[0:np_, :], axis=AX.X,
                                        op=ALU.add)
                vrows[g] = vrow

            # output layout: [0:8] ab, [8] bb, [9:17] aa
            nc.sync.dma_start(vals_d[0:NA], vrows[0][0:NA, :])
            nc.sync.dma_start(vals_d[NA:NPROB], vrows[1][0:NB, :])

    if not nc.is_finalized():
        # All ScalarE funcs used here (Exp, Ln, Identity, Copy) live in the
        # single act-table set "natural_log_exp_and_others"; hide the other
        # sets (preserving list positions = act_func_set_id) so the table
        # load pass emits one load instead of thrashing between
        # exp_and_others and natural_log every half-update.
        import concourse.bacc as bacc_mod
        _orig_gat = bacc_mod.get_activation_tables
        def _one_set_gat(arch):
            t = _orig_gat(arch)
            return {name: (fns if name == "natural_log_exp_and_others" else set())
                    for name, fns in t.items()}
        bacc_mod.get_activation_tables = _one_set_gat
        try:
            nc.finalize()
        finally:
            bacc_mod.get_activation_tables = _orig_gat
    return nc


def _log_softmax(x):
    m = x.max(axis=-1, keepdims=True)
    e = np.exp(x - m)
    return (x - m) - np.log(e.sum(axis=-1, keepdims=True))


def prep_in_maps(fea, y, W, bW, anchor_tab):
    fea = np.ascontiguousarray(np.asarray(fea, dtype=np.float32))
    y = np.asarray(y).astype(np.int64)
    W = np.asarray(W, dtype=np.float32)
    bW = np.asarray(bW, dtype=np.float32)
    anchor_tab = np.asarray(anchor_tab, dtype=np.float32)

    logb_cls = _log_softmax(anchor_tab).astype(np.float32)      # [5, D]
    bW_g = bW[y]                                                # [B, D]
    onehot = np.zeros((B, NUM_CLASSES), dtype=np.float32)
    onehot[np.arange(B), y] = 1.0

    in_maps = []
    for k in range(NCORES):
        sl = slice(k * DSL, (k + 1) * DSL)
        logbs = np.empty((SAMP + 1, D), dtype=np.float32)
        for p in range(SAMP):
            logbs[p] = logb_cls[y[k * SAMP + p]]
        logbs[SAMP] = logb_cls[k] if k < NUM_CLASSES else logb_cls[0]
        in_maps.append({
            "fea": fea,
            "wsl": np.ascontiguousarray(W[:, sl, :]),
            "bwsl": np.ascontiguousarray(bW_g[:, sl]),
            "onehot": onehot,
            "logbs": logbs,
        })
    return in_maps


def postprocess(vals_per_core, y):
    y = np.asarray(y).astype(np.int64)
    V_ab = np.empty(B, dtype=np.float32)
    V_aa = np.empty(B, dtype=np.float32)
    V_bb = np.empty(NUM_CLASSES, dtype=np.float32)
    for k in range(NCORES):
        v = np.asarray(vals_per_core[k]).reshape(-1)
        V_ab[k * SAMP:(k + 1) * SAMP] = v[0:SAMP]
        V_aa[k * SAMP:(k + 1) * SAMP] = v[NA:NPROB]
        if k < NUM_CLASSES:
            V_bb[k] = v[SAMP]
    losses = (2.0 * V_ab - V_aa - V_bb[y]).astype(np.float32)
    z = -losses * np.float32(TEMP)
    zm = z.max()
    e = np.exp(z - zm)
    weights = (e / e.sum()).astype(np.float32)
    loss = np.float32(np.sum(losses * weights))
    return loss, weights


_CACHE = {}


def kernel(fea, y, W, bW, anchor_tab):
    from concourse.bass_utils import run_bass_kernel_spmd

    if "nc" not in _CACHE:
        _CACHE["nc"] = _build_program(ITERS)
    nc = _CACHE["nc"]

    in_maps = prep_in_maps(fea, y, W, bW, anchor_tab)
    res = run_bass_kernel_spmd(nc, in_maps, list(range(NCORES)))
    vals = [res.results[k]["values"] for k in range(NCORES)]
    return postprocess(vals, y)


# revision 29
# speedup vs baseline: 1.0044x; 1.0044x over previous
"""Trainium2 Bass kernel for the Sinkhorn-OT loss problem.

Math summary (mirrors the reference):
  mapped = einsum('bf,bdf->bd', fea, W[y]) + bW[y];  a = softmax(mapped)
  b = softmax(anchor_tab[y]);  M_ij = (i-j)^2 on D=1024, eps=0.05
  ot(x, y) = 50-iteration log-domain Sinkhorn value
  losses = 2*ot(a,b) - ot(a,a) - ot(b,b);  weights = softmax(-losses)
  loss = sum(losses*weights)

Key structural facts exploited:
  * The Gibbs kernel exp(-(i-j)^2/eps) decays by e^-20 per unit distance, so
    every logsumexp over D is exactly banded with radius 3 in f32 (terms at
    |d|=4 sit >= 40 below the row max - invisible at f32 precision).
  * ot(b,b) has only 5 distinct instances (one per class).
  * The warm-start stabilizer (previous iteration's lse output) keeps all
    exp arguments in [-inf, ~7]; no per-row max pass is needed.

Distribution over the 8 cores:
  Phase A: D-parallel matmul. Core k holds W[:, 128k:128k+128, :], computes
    that D-slice of mapped for all 64 samples on the TensorEngine, then an
    AllToAll redistributes slices so core k owns full rows for samples
    8k..8k+8.
  Phase B: problem-parallel Sinkhorn. Each core runs 17 independent solves
    in two interleaved groups (A: 8 ab + 1 bb, B: 8 aa) so the two serial
    update chains fill each other's pipeline gaps. Each problem's D=1024
    axis is folded as [128 partitions x 8 cols] plus a 3-wide halo
    maintained by TensorEngine shift-matmuls.
  Host: assembles the 133 values, forms losses/weights/loss (O(64) work).

Problem order in the "values" output: [0:8] ab, [8] bb, [9:17] aa.
"""

import numpy as np

NUM_CLASSES = 5
FEAT = 2048
D = 1024
B = 64
EPS = 0.05
ITERS = 50
TEMP = 1.0

R = 3                 # band radius
HW = 8 + 2 * R        # halo'd fold width (14)
NCORES = 8
SAMP = B // NCORES    # samples per core (8)
NPROB = 2 * SAMP + 1  # problems per core (17)
DSL = D // NCORES     # D-slice per core (128)
FOLD = D // 128       # fold width (8)
NEG = -1.0e30
import os
EXP_SPLIT = int(os.environ.get('EXP_SPLIT', '1'))
HALO_DVE = int(os.environ.get('HALO_DVE', '0'))
NA = SAMP + 1         # group A: 8 ab + 1 bb
NB = SAMP             # group B: 8 aa


def _build_program(iters=ITERS, collective=True):
    import concourse.bass as bass
    import concourse.bacc as bacc
    import concourse.tile as tile
    from concourse import mybir

    f32 = mybir.dt.float32
    AF = mybir.ActivationFunctionType
    ALU = mybir.AluOpType
    AX = mybir.AxisListType

    nc = bacc.Bacc("TRN2", target_bir_lowering=False, debug=False)

    fea_d = nc.declare_dram_parameter("fea", [B, FEAT], f32, isOutput=False)
    wsl_d = nc.declare_dram_parameter("wsl", [NUM_CLASSES, DSL, FEAT], f32, isOutput=False)
    bwsl_d = nc.declare_dram_parameter("bwsl", [B, DSL], f32, isOutput=False)
    onehot_d = nc.declare_dram_parameter("onehot", [B, NUM_CLASSES], f32, isOutput=False)
    logbs_d = nc.declare_dram_parameter("logbs", [SAMP + 1, D], f32, isOutput=False)
    vals_d = nc.declare_dram_parameter("values", [NPROB], f32, isOutput=True)

    ident_d = nc.inline_tensor(np.eye(128, dtype=np.float32), name="ident")
    # shift-down: out[q] = in[q-1]  (SD[k, m] = 1 iff k = m-1)
    sd_d = nc.inline_tensor(np.eye(128, k=1, dtype=np.float32), name="sdn")
    # shift-up: out[q] = in[q+1]  (SU[k, m] = 1 iff k = m+1)
    su_d = nc.inline_tensor(np.eye(128, k=-1, dtype=np.float32), name="sup")
    ones_d = nc.inline_tensor(np.ones((1, 128), dtype=np.float32), name="ones1")
    bneg0_np = np.zeros((128, 1), dtype=np.float32); bneg0_np[0, 0] = NEG
    bneg127_np = np.zeros((128, 1), dtype=np.float32); bneg127_np[127, 0] = NEG
    bneg0_d = nc.inline_tensor(bneg0_np, name="bneg0")
    bneg127_d = nc.inline_tensor(bneg127_np, name="bneg127")
    # halo boundary mask: cols 0:3 NEG at partition 0, cols 3:6 NEG at partition 127
    hm_np = np.zeros((128, 6), dtype=np.float32)
    hm_np[0, 0:3] = NEG; hm_np[127, 3:6] = NEG
    hmask_d = nc.inline_tensor(hm_np, name="hmask")

    with tile.TileContext(nc) as tc:
        with (
            tc.tile_pool(name="consts", bufs=1) as consts,
            tc.tile_pool(name="pha", bufs=2) as pha,
            tc.tile_pool(name="state", bufs=1) as state,
            tc.tile_pool(name="scratch", bufs=2) as scratch,
            tc.tile_pool(name="psum_mm", bufs=2, space="PSUM") as psum_mm,
            tc.tile_pool(name="psum_tr", bufs=2, space="PSUM") as psum_tr,
            tc.tile_pool(name="psum_h", bufs=1, space="PSUM") as psum_h,
            tc.tile_pool(name="dram", bufs=1, space="DRAM") as dram,
        ):
            # ---------- constants to SBUF ----------
            ident = consts.tile([128, 128], f32, tag="ident")
            nc.sync.dma_start(ident, ident_d[:, :])
            sdn = consts.tile([128, 128], f32, tag="sdn")
            nc.sync.dma_start(sdn, sd_d[:, :])
            sup = consts.tile([128, 128], f32, tag="sup")
            nc.sync.dma_start(sup, su_d[:, :])
            ones1 = consts.tile([1, 128], f32, tag="ones1")
            nc.sync.dma_start(ones1, ones_d[:, :])
            onehot = consts.tile([B, NUM_CLASSES], f32, tag="onehot")
            nc.sync.dma_start(onehot, onehot_d[:, :])
            bwsl = consts.tile([B, DSL], f32, tag="bwsl")
            nc.sync.dma_start(bwsl, bwsl_d[:, :])
            bneg0 = consts.tile([128, 1], f32, tag="bneg0")
            nc.sync.dma_start(bneg0, bneg0_d[:, :])
            bneg127 = consts.tile([128, 1], f32, tag="bneg127")
            nc.sync.dma_start(bneg127, bneg127_d[:, :])
            hmask = consts.tile([128, 6], f32, tag="hmask")
            nc.sync.dma_start(hmask, hmask_d[:, :])

            # per-partition bias constants for activation (c_d = -20*d^2)
            cbias = {}
            for v in sorted({-20.0 * d * d for d in range(0, R + 1)}):
                t = consts.tile([128, 1], f32, tag=f"cb{int(-v)}")
                nc.vector.memset(t, v)
                cbias[v] = t

            # ---------- Phase A: mapped D-slice for all 64 samples ----------
            fea_sb = consts.tile([B, FEAT], f32, tag="fea")
            nc.sync.dma_start(fea_sb, fea_d[:, :])

            feaT = consts.tile([128, 16, B], f32, tag="feaT")
            for j in range(16):
                pt = psum_tr.tile([128, B], f32, tag="tr")
                nc.tensor.transpose(pt, fea_sb[:, j * 128:(j + 1) * 128], ident[0:B, 0:B])
                nc.scalar.copy(feaT[:, j, :], pt)

            mapped = state.tile([B, DSL], f32, tag="mapped")
            for c in range(NUM_CLASSES):
                wc = pha.tile([128, FEAT], f32, tag="wc")
                for h in range(4):
                    nc.sync.dma_start(wc[:, h * 512:(h + 1) * 512],
                                      wsl_d[c, :, h * 512:(h + 1) * 512])
                wcT = pha.tile([128, 16, DSL], f32, tag="wcT")
                for j in range(16):
                    ptw = psum_tr.tile([128, 128], f32, tag="tr")
                    nc.tensor.transpose(ptw, wc[:, j * 128:(j + 1) * 128], ident)
                    if j % 2 == 0:
                        nc.vector.tensor_copy(wcT[:, j, :], ptw)
                    else:
                        nc.scalar.copy(wcT[:, j, :], ptw)
                pmm = psum_mm.tile([B, DSL], f32, tag="pmm")
                for j in range(16):
                    nc.tensor.matmul(pmm, lhsT=feaT[:, j, :], rhs=wcT[:, j, :],
                                     start=(j == 0), stop=(j == 15))
                if c == 0:
                    nc.vector.tensor_scalar_mul(mapped, pmm, onehot[:, 0:1])
                else:
                    nc.vector.scalar_tensor_tensor(
                        out=mapped, in0=pmm, scalar=onehot[:, c:c + 1], in1=mapped,
                        op0=ALU.mult, op1=ALU.add)
            nc.vector.tensor_add(mapped, mapped, bwsl)

            # ---------- AllToAll: D-slices -> per-core full rows ----------
            ag_in = dram.tile([B, DSL], f32, tag="ag_in")
            ag_out = dram.tile([B, DSL], f32, tag="ag_out")
            nc.sync.dma_start(ag_in, mapped)
            if collective:
                nc.gpsimd.collective_compute(
                    "AllToAll", ALU.bypass,
                    replica_groups=[list(range(NCORES))],
                    ins=[ag_in[:, :].opt()], outs=[ag_out[:, :].opt()])
            else:
                # single-core timing variant: plain copy stands in for AllToAll
                nc.sync.dma_start(ag_out, ag_in[:, :])

            # ag_out[j*8 + p, :] = D-slice j of sample (8*core + p).
            # fold sample p's full row into [128, 8]: partition q holds D[8q..8q+8)
            mraw = state.tile([128, SAMP, FOLD], f32, tag="mraw")
            agv = ag_out[:, :].rearrange("(j p) (q c) -> j p q c", p=SAMP, c=FOLD)
            for p in range(SAMP):
                nc.sync.dma_start(mraw[:, p, :], agv[:, p, :, :])

            # ---------- loga = mraw - lse_D(mraw) per sample ----------
            def cross_partition_chain(src, nprob, op_alu):
                pr = scratch.tile([128, nprob], f32, tag="cp_pr")
                nc.vector.tensor_reduce(pr, src, axis=AX.X, op=op_alu)
                tp = psum_tr.tile([nprob, 128], f32, tag="tr")
                nc.tensor.transpose(tp, pr, ident)
                red = scratch.tile([nprob, 1], f32, tag="cp_red")
                nc.vector.tensor_reduce(red, tp, axis=AX.X, op=op_alu)
                return red

            def bcast_over_parts(col, nprob):
                tpc = psum_tr.tile([1, nprob], f32, tag="tr")
                nc.tensor.transpose(tpc, col, ident[0:nprob, 0:nprob])
                row = scratch.tile([1, nprob], f32, tag="bc_row")
                nc.scalar.copy(row, tpc)
                bc = psum_tr.tile([128, nprob], f32, tag="tr")
                nc.tensor.matmul(bc, lhsT=ones1, rhs=row, start=True, stop=True)
                return bc

            # mapped ~ N(0,1)-scale (|x| < ~10), so exp needs no max-stabilizer:
            # Za = ln(sum exp(mapped)) directly.
            e_e = scratch.tile([128, SAMP, FOLD], f32, tag="e_e")
            nc.scalar.activation(e_e, mraw, AF.Exp)
            ss_col = cross_partition_chain(e_e, SAMP, ALU.add)
            za = scratch.tile([SAMP, 1], f32, tag="za")
            nc.scalar.activation(za, ss_col, AF.Ln)
            bz = bcast_over_parts(za, SAMP)
            loga = state.tile([128, SAMP, FOLD], f32, tag="loga")
            nc.vector.tensor_sub(loga, mraw, bz[:, :, None].broadcast_to([128, SAMP, FOLD]))

            # ---------- marginals per group ----------
            # group A (9 probs): 0..7 ab (LA=loga_p, LB=logb_p), 8 bb (LA=LB=logb_cls)
            # group B (8 probs): aa (LA=LB=loga_p)
            NP = {0: NA, 1: NB}
            LA = {}; LB = {}
            LA[0] = state.tile([128, NA, FOLD], f32, tag="LA_A", name="LA_A")
            LB[0] = state.tile([128, NA, FOLD], f32, tag="LB_A", name="LB_A")
            LA[1] = state.tile([128, NB, FOLD], f32, tag="LA_B", name="LA_B")
            LB[1] = state.tile([128, NB, FOLD], f32, tag="LB_B", name="LB_B")
            nc.scalar.copy(LA[0][:, 0:SAMP, :], loga)
            nc.scalar.copy(LA[1][:, :, :], loga)
            nc.scalar.copy(LB[1][:, :, :], loga)
            for p in range(SAMP):
                nc.sync.dma_start(
                    LB[0][:, p, :],
                    logbs_d[p, :].rearrange("(q c) -> q c", c=FOLD))
            nc.sync.dma_start(
                LA[0][:, SAMP, :],
                logbs_d[SAMP, :].rearrange("(q c) -> q c", c=FOLD))
            nc.sync.dma_start(
                LB[0][:, SAMP, :],
                logbs_d[SAMP, :].rearrange("(q c) -> q c", c=FOLD))

            # ---------- Sinkhorn state (per group) ----------
            SFh = {}; SGh = {}; Mf = {}; Mg = {}; Tb = {}; Eb = {}
            acc = {}; lnacc = {}
            for g in (0, 1):
                s = "AB"[g]
                SFh[g] = state.tile([128, NP[g], HW], f32, tag=f"SFh{s}", name=f"SFh{s}")
                SGh[g] = state.tile([128, NP[g], HW], f32, tag=f"SGh{s}", name=f"SGh{s}")
                Mf[g] = state.tile([128, NP[g], FOLD], f32, tag=f"Mf{s}", name=f"Mf{s}")
                Mg[g] = state.tile([128, NP[g], FOLD], f32, tag=f"Mg{s}", name=f"Mg{s}")
                # band-slot order: [0]=d0, [1]=-1, [2]=+1, [3]=-2, [4]=+2, [5]=-3, [6]=+3
                # scratch double-buffered by update parity (f/g) to kill WAR stalls
                for par in (0, 1):
                    Tb[g, par] = state.tile([128, NP[g], FOLD, 2 * R + 1], f32, tag=f"Tb{s}{par}", name=f"Tb{s}{par}")
                    Eb[g, par] = state.tile([128, NP[g], FOLD, 2 * R + 1], f32, tag=f"Eb{s}{par}", name=f"Eb{s}{par}")
                    acc[g, par] = state.tile([128, NP[g], FOLD], f32, tag=f"acc{s}{par}", name=f"acc{s}{par}")
                    lnacc[g, par] = state.tile([128, NP[g], FOLD], f32, tag=f"lnacc{s}{par}", name=f"lnacc{s}{par}")
                nc.vector.memset(Mf[g], 0.0)
                nc.vector.memset(Mg[g], 0.0)
                nc.vector.memset(SFh[g], 0.0)
                nc.vector.memset(SGh[g], 0.0)

            def halo_fill(buf, g):
                np_ = NP[g]
                # both halos via one PSUM tile + one 2-range DVE add.
                # left halo: buf[q, :, 0:R] = buf[q-1, :, FOLD:FOLD+R]
                # right halo: buf[q, :, R+FOLD:HW] = buf[q+1, :, R:2R]
                # boundary partitions forced to NEG by the hmask tile.
                if HALO_DVE:
                    ph = psum_h.tile([128, 2, NA, R], f32, tag=f"halo{'AB'[g]}")
                    nc.tensor.matmul(ph[:, 0, 0:np_, :], lhsT=sdn,
                                     rhs=buf[:, :, FOLD:FOLD + R], start=True, stop=True)
                    nc.tensor.matmul(ph[:, 1, 0:np_, :], lhsT=sup,
                                     rhs=buf[:, :, R:2 * R], start=True, stop=True)
                    # one DVE add writes both halo col-ranges: dims (side, prob, col)
                    bb = buf[:, :, 0:R]
                    halo_out = bass.AP(tensor=bb.tensor, offset=bb.offset,
                                       ap=[bb.ap[0], [R + FOLD, 2], bb.ap[1], [1, R]])
                    ph_v = ph[:, :, 0:np_, :]
                    hs = hmask[:, :]
                    hm_v = bass.AP(tensor=hs.tensor, offset=hs.offset,
                                   ap=[hs.ap[0], [R, 2], [0, np_], [1, R]])
                    nc.vector.tensor_add(halo_out, ph_v, hm_v)
                else:
                    pl = psum_h.tile([128, NA, R], f32, tag=f"hl{'AB'[g]}")
                    nc.tensor.matmul(pl[:, 0:np_, :], lhsT=sdn,
                                     rhs=buf[:, :, FOLD:FOLD + R], start=True, stop=True)
                    nc.scalar.add(buf[:, :, 0:R], pl[:, 0:np_, :], bneg0[:, 0:1])
                    pr_ = psum_h.tile([128, NA, R], f32, tag=f"hr{'AB'[g]}")
                    nc.tensor.matmul(pr_[:, 0:np_, :], lhsT=sup,
                                     rhs=buf[:, :, R:2 * R], start=True, stop=True)
                    nc.scalar.add(buf[:, :, R + FOLD:HW], pr_[:, 0:np_, :],
                                  bneg127[:, 0:1])

            for g in (0, 1):
                nc.scalar.copy(SGh[g][:, :, R:R + FOLD], LB[g])
                nc.scalar.copy(SFh[g][:, :, R:R + FOLD], LA[g])
                halo_fill(SGh[g], g)

            def pair_ap(buf, k, np_):
                """[128, np, FOLD, 2] view of buf: pair (-k, +k) windows."""
                base = buf[:, :, R - k:R - k + FOLD]
                return bass.AP(tensor=base.tensor, offset=base.offset,
                               ap=[*base.ap, [2 * k, 2]])

            def bcast2_ap(t):
                """append a step-0 pair dim to a [128, np, FOLD] view."""
                return bass.AP(tensor=t.tensor, offset=t.offset,
                               ap=[*t.ap, [0, 2]])

            def half_update(g, src, dst, M, LAd, par):
                T, E = Tb[g, par], Eb[g, par]
                np_ = NP[g]
                # pair windows via 4D tensor_sub (walrus-legal); c_d goes into
                # the per-pair Exp bias (c identical for +-d).
                nc.vector.tensor_sub(T[:, :, :, 0], src[:, :, R:R + FOLD], M)
                for k in (1, 2, 3):
                    nc.vector.tensor_sub(
                        T[:, :, :, 2 * k - 1:2 * k + 1], pair_ap(src, k, np_),
                        bcast2_ap(M[:, :, :]))
                nc.scalar.activation(E[:, :, :, 0], T[:, :, :, 0], AF.Exp)
                for k in (1, 2, 3):
                    nc.scalar.activation(
                        E[:, :, :, 2 * k - 1:2 * k + 1], T[:, :, :, 2 * k - 1:2 * k + 1],
                        AF.Exp, bias=cbias[-20.0 * k * k][:, 0:1])
                nc.vector.tensor_reduce(acc[g, par], E, axis=AX.X, op=ALU.add)
                nc.scalar.activation(lnacc[g, par], acc[g, par], AF.Ln)
                # dst center: LA - M_new == dst_prev - lnacc (dst was LA - M_old);
                # keeps the halo shift off the M-update dependency
                nc.vector.tensor_sub(dst[:, :, R:R + FOLD],
                                     dst[:, :, R:R + FOLD], lnacc[g, par])
                nc.gpsimd.tensor_add(M, M, lnacc[g, par])
                halo_fill(dst, g)

            for _t in range(iters):
                half_update(0, SGh[0], SFh[0], Mf[0], LA[0], 0)
                half_update(1, SGh[1], SFh[1], Mf[1], LA[1], 0)
                half_update(0, SFh[0], SGh[0], Mg[0], LB[0], 1)
                half_update(1, SFh[1], SGh[1], Mg[1], LB[1], 1)

            # ---------- value: V = sum_{i,d} exp(SF_i+SG_{i+d}+c_d)(F_i+G_{i+d}) ----------
            vrows = {}
            for g in (0, 1):
                np_ = NP[g]
                Ft = scratch.tile([128, NA, FOLD], f32, tag="Ft", name="Ft")[:, 0:np_, :]
                nc.vector.tensor_sub(Ft, SFh[g][:, :, R:R + FOLD], LA[g])
                nc.vector.tensor_scalar_mul(Ft, Ft, float(EPS))
                Gh = scratch.tile([128, NA, HW], f32, tag="Gh", name="Gh")[:, 0:np_, :]
                nc.vector.tensor_sub(Gh[:, :, R:R + FOLD], SGh[g][:, :, R:R + FOLD], LB[g])
                nc.vector.tensor_scalar_mul(
                    Gh[:, :, R:R + FOLD], Gh[:, :, R:R + FOLD], float(EPS))
                halo_fill(Gh, g)

                s1 = scratch.tile([128, NA, FOLD], f32, tag="s1", name="s1")[:, 0:np_, :]
                pd_ = scratch.tile([128, NA, FOLD], f32, tag="pd", name="pd")[:, 0:np_, :]
                s2 = scratch.tile([128, NA, FOLD], f32, tag="s2", name="s2")[:, 0:np_, :]
                vacc = scratch.tile([128, NA, FOLD], f32, tag="vacc", name="vacc")[:, 0:np_, :]
                for di, d in enumerate(range(-R, R + 1)):
                    nc.vector.tensor_add(
                        s1, SFh[g][:, :, R:R + FOLD], SGh[g][:, :, R + d:R + d + FOLD])
                    nc.scalar.activation(pd_, s1, AF.Exp,
                                         bias=cbias[-20.0 * d * d][:, 0:1])
                    nc.vector.tensor_add(s2, Ft, Gh[:, :, R + d:R + d + FOLD])
                    if di == 0:
                        nc.vector.tensor_mul(vacc, pd_, s2)
                    else:
                        nc.vector.tensor_mul(s2, pd_, s2)
                        nc.vector.tensor_add(vacc, vacc, s2)

                vsum = scratch.tile([128, NA], f32, tag="vsum", name="vsum")[:, 0:np_]
                nc.vector.tensor_reduce(vsum, vacc, axis=AX.X, op=ALU.add)
                tv = psum_tr.tile([NA, 128], f32, tag="tr")
                nc.tensor.transpose(tv[0:np_, :], vsum, ident)
                vrow = scratch.tile([NA, 1], f32, tag=f"vrow{'AB'[g]}", name=f"vrow{'AB'[g]}")
                nc.vector.tensor_reduce(vrow[0:np_, :], tv[0:np_, :], axis=AX.X,
                                        op=ALU.add)
                vrows[g] = vrow

            # output layout: [0:8] ab, [8] bb, [9:17] aa
            nc.sync.dma_start(vals_d[0:NA], vrows[0][0:NA, :])
            nc.sync.dma_start(vals_d[NA:NPROB], vrows[1][0:NB, :])

    if not nc.is_finalized():
        # All ScalarE funcs used here (Exp, Ln, Identity, Copy) live in the
        # single act-table set "natural_log_exp_and_others"; hide the other
        # sets (preserving list positions = act_func_set_id) so the table
        # load pass emits one load instead of thrashing between
        # exp_and_others and natural_log every half-update.
        import concourse.bacc as bacc_mod
        _orig_gat = bacc_mod.get_activation_tables
        def _one_set_gat(arch):
            t = _orig_gat(arch)
            return {name: (fns if name == "natural_log_exp_and_others" else set())
                    for name, fns in t.items()}
        bacc_mod.get_activation_tables = _one_set_gat
        try:
            nc.finalize()
        finally:
            bacc_mod.get_activation_tables = _orig_gat
    return nc


def _log_softmax(x):
    m = x.max(axis=-1, keepdims=True)
    e = np.exp(x - m)
    return (x - m) - np.log(e.sum(axis=-1, keepdims=True))


def prep_in_maps(fea, y, W, bW, anchor_tab):
    fea = np.ascontiguousarray(np.asarray(fea, dtype=np.float32))
    y = np.asarray(y).astype(np.int64)
    W = np.asarray(W, dtype=np.float32)
    bW = np.asarray(bW, dtype=np.float32)
    anchor_tab = np.asarray(anchor_tab, dtype=np.float32)

    logb_cls = _log_softmax(anchor_tab).astype(np.float32)      # [5, D]
    bW_g = bW[y]                                                # [B, D]
    onehot = np.zeros((B, NUM_CLASSES), dtype=np.float32)
    onehot[np.arange(B), y] = 1.0

    in_maps = []
    for k in range(NCORES):
        sl = slice(k * DSL, (k + 1) * DSL)
        logbs = np.empty((SAMP + 1, D), dtype=np.float32)
        for p in range(SAMP):
            logbs[p] = logb_cls[y[k * SAMP + p]]
        logbs[SAMP] = logb_cls[k] if k < NUM_CLASSES else logb_cls[0]
        in_maps.append({
            "fea": fea,
            "wsl": np.ascontiguousarray(W[:, sl, :]),
            "bwsl": np.ascontiguousarray(bW_g[:, sl]),
            "onehot": onehot,
            "logbs": logbs,
        })
    return in_maps


def postprocess(vals_per_core, y):
    y = np.asarray(y).astype(np.int64)
    V_ab = np.empty(B, dtype=np.float32)
    V_aa = np.empty(B, dtype=np.float32)
    V_bb = np.empty(NUM_CLASSES, dtype=np.float32)
    for k in range(NCORES):
        v = np.asarray(vals_per_core[k]).reshape(-1)
        V_ab[k * SAMP:(k + 1) * SAMP] = v[0:SAMP]
        V_aa[k * SAMP:(k + 1) * SAMP] = v[NA:NPROB]
        if k < NUM_CLASSES:
            V_bb[k] = v[SAMP]
    losses = (2.0 * V_ab - V_aa - V_bb[y]).astype(np.float32)
    z = -losses * np.float32(TEMP)
    zm = z.max()
    e = np.exp(z - zm)
    weights = (e / e.sum()).astype(np.float32)
    loss = np.float32(np.sum(losses * weights))
    return loss, weights


_CACHE = {}


def kernel(fea, y, W, bW, anchor_tab):
    from concourse.bass_utils import run_bass_kernel_spmd

    if "nc" not in _CACHE:
        _CACHE["nc"] = _build_program(ITERS)
    nc = _CACHE["nc"]

    in_maps = prep_in_maps(fea, y, W, bW, anchor_tab)
    res = run_bass_kernel_spmd(nc, in_maps, list(range(NCORES)))
    vals = [res.results[k]["values"] for k in range(NCORES)]
    return postprocess(vals, y)


# revision 35
# speedup vs baseline: 1.0378x; 1.0332x over previous
"""Trainium2 Bass kernel for the Sinkhorn-OT loss problem.

Math summary (mirrors the reference):
  mapped = einsum('bf,bdf->bd', fea, W[y]) + bW[y];  a = softmax(mapped)
  b = softmax(anchor_tab[y]);  M_ij = (i-j)^2 on D=1024, eps=0.05
  ot(x, y) = 50-iteration log-domain Sinkhorn value
  losses = 2*ot(a,b) - ot(a,a) - ot(b,b);  weights = softmax(-losses)
  loss = sum(losses*weights)

Key structural facts exploited:
  * The Gibbs kernel exp(-(i-j)^2/eps) decays by e^-20 per unit distance, so
    every logsumexp over D is exactly banded with radius 3 in f32 (terms at
    |d|=4 sit >= 40 below the row max - invisible at f32 precision).
  * ot(b,b) has only 5 distinct instances (one per class).
  * The warm-start stabilizer (previous iteration's lse output) keeps all
    exp arguments in [-inf, ~7]; no per-row max pass is needed.

Distribution over the 8 cores:
  Phase A: D-parallel matmul. Core k holds W[:, 128k:128k+128, :], computes
    that D-slice of mapped for all 64 samples on the TensorEngine, then an
    AllToAll redistributes slices so core k owns full rows for samples
    8k..8k+8.
  Phase B: problem-parallel Sinkhorn. Each core runs 17 independent solves
    in two interleaved groups (A: 8 ab + 1 bb, B: 8 aa) so the two serial
    update chains fill each other's pipeline gaps. Each problem's D=1024
    axis is folded as [128 partitions x 8 cols] plus a 3-wide halo
    maintained by TensorEngine shift-matmuls.
  Host: assembles the 133 values, forms losses/weights/loss (O(64) work).

Problem order in the "values" output: [0:8] ab, [8] bb, [9:17] aa.
"""

import numpy as np

NUM_CLASSES = 5
FEAT = 2048
D = 1024
B = 64
EPS = 0.05
ITERS = 50
TEMP = 1.0

R = 3                 # band radius
HW = 8 + 2 * R        # halo'd fold width (14)
NCORES = 8
SAMP = B // NCORES    # samples per core (8)
NPROB = 2 * SAMP + 1  # problems per core (17)
DSL = D // NCORES     # D-slice per core (128)
FOLD = D // 128       # fold width (8)
NEG = -1.0e30
import os
EXP_SPLIT = int(os.environ.get('EXP_SPLIT', '1'))
HALO_DVE = int(os.environ.get('HALO_DVE', '0'))
NA = SAMP + 1         # group A: 8 ab + 1 bb
NB = SAMP             # group B: 8 aa


def _build_program(iters=ITERS, collective=True):
    import concourse.bass as bass
    import concourse.bacc as bacc
    import concourse.tile as tile
    from concourse import mybir

    f32 = mybir.dt.float32
    AF = mybir.ActivationFunctionType
    ALU = mybir.AluOpType
    AX = mybir.AxisListType

    nc = bacc.Bacc("TRN2", target_bir_lowering=False, debug=False)

    fea_d = nc.declare_dram_parameter("fea", [B, FEAT], f32, isOutput=False)
    wsl_d = nc.declare_dram_parameter("wsl", [NUM_CLASSES, FEAT, DSL], f32, isOutput=False)
    bwsl_d = nc.declare_dram_parameter("bwsl", [B, DSL], f32, isOutput=False)
    onehot_d = nc.declare_dram_parameter("onehot", [B, NUM_CLASSES], f32, isOutput=False)
    logbs_d = nc.declare_dram_parameter("logbs", [SAMP + 1, D], f32, isOutput=False)
    vals_d = nc.declare_dram_parameter("values", [NPROB], f32, isOutput=True)

    ident_d = nc.inline_tensor(np.eye(128, dtype=np.float32), name="ident")
    # shift-down: out[q] = in[q-1]  (SD[k, m] = 1 iff k = m-1)
    sd_d = nc.inline_tensor(np.eye(128, k=1, dtype=np.float32), name="sdn")
    # shift-up: out[q] = in[q+1]  (SU[k, m] = 1 iff k = m+1)
    su_d = nc.inline_tensor(np.eye(128, k=-1, dtype=np.float32), name="sup")
    ones_d = nc.inline_tensor(np.ones((1, 128), dtype=np.float32), name="ones1")
    bneg0_np = np.zeros((128, 1), dtype=np.float32); bneg0_np[0, 0] = NEG
    bneg127_np = np.zeros((128, 1), dtype=np.float32); bneg127_np[127, 0] = NEG
    bneg0_d = nc.inline_tensor(bneg0_np, name="bneg0")
    bneg127_d = nc.inline_tensor(bneg127_np, name="bneg127")
    # halo boundary mask: cols 0:3 NEG at partition 0, cols 3:6 NEG at partition 127
    hm_np = np.zeros((128, 6), dtype=np.float32)
    hm_np[0, 0:3] = NEG; hm_np[127, 3:6] = NEG
    hmask_d = nc.inline_tensor(hm_np, name="hmask")

    with tile.TileContext(nc) as tc:
        with (
            tc.tile_pool(name="consts", bufs=1) as consts,
            tc.tile_pool(name="pha", bufs=2) as pha,
            tc.tile_pool(name="state", bufs=1) as state,
            tc.tile_pool(name="scratch", bufs=2) as scratch,
            tc.tile_pool(name="psum_mm", bufs=2, space="PSUM") as psum_mm,
            tc.tile_pool(name="psum_tr", bufs=2, space="PSUM") as psum_tr,
            tc.tile_pool(name="psum_h", bufs=1, space="PSUM") as psum_h,
            tc.tile_pool(name="dram", bufs=1, space="DRAM") as dram,
        ):
            # ---------- constants to SBUF ----------
            ident = consts.tile([128, 128], f32, tag="ident")
            nc.sync.dma_start(ident, ident_d[:, :])
            sdn = consts.tile([128, 128], f32, tag="sdn")
            nc.sync.dma_start(sdn, sd_d[:, :])
            sup = consts.tile([128, 128], f32, tag="sup")
            nc.sync.dma_start(sup, su_d[:, :])
            ones1 = consts.tile([1, 128], f32, tag="ones1")
            nc.sync.dma_start(ones1, ones_d[:, :])
            onehot = consts.tile([B, NUM_CLASSES], f32, tag="onehot")
            nc.sync.dma_start(onehot, onehot_d[:, :])
            bwsl = consts.tile([B, DSL], f32, tag="bwsl")
            nc.sync.dma_start(bwsl, bwsl_d[:, :])
            bneg0 = consts.tile([128, 1], f32, tag="bneg0")
            nc.sync.dma_start(bneg0, bneg0_d[:, :])
            bneg127 = consts.tile([128, 1], f32, tag="bneg127")
            nc.sync.dma_start(bneg127, bneg127_d[:, :])
            hmask = consts.tile([128, 6], f32, tag="hmask")
            nc.sync.dma_start(hmask, hmask_d[:, :])

            # per-partition bias constants for activation (c_d = -20*d^2)
            cbias = {}
            for v in sorted({-20.0 * d * d for d in range(0, R + 1)}):
                t = consts.tile([128, 1], f32, tag=f"cb{int(-v)}")
                nc.vector.memset(t, v)
                cbias[v] = t

            # ---------- Phase A: mapped D-slice for all 64 samples ----------
            fea_sb = consts.tile([B, FEAT], f32, tag="fea")
            nc.sync.dma_start(fea_sb, fea_d[:, :])

            feaT = consts.tile([128, 16, B], f32, tag="feaT")
            for j in range(16):
                pt = psum_tr.tile([128, B], f32, tag="tr")
                nc.tensor.transpose(pt, fea_sb[:, j * 128:(j + 1) * 128], ident[0:B, 0:B])
                nc.scalar.copy(feaT[:, j, :], pt)

            mapped = state.tile([B, DSL], f32, tag="mapped")
            for c in range(NUM_CLASSES):
                # W arrives host-pre-transposed as [C, FEAT, DSL]: DMA the
                # [feat-chunk, d] tiles straight into matmul-ready layout.
                wcT = pha.tile([128, 16, DSL], f32, tag="wcT")
                wv = wsl_d[c, :, :].rearrange("(j p) d -> p j d", p=128)
                for h in range(4):
                    nc.sync.dma_start(wcT[:, h * 4:(h + 1) * 4, :],
                                      wv[:, h * 4:(h + 1) * 4, :])
                pmm = psum_mm.tile([B, DSL], f32, tag="pmm")
                for j in range(16):
                    nc.tensor.matmul(pmm, lhsT=feaT[:, j, :], rhs=wcT[:, j, :],
                                     start=(j == 0), stop=(j == 15))
                if c == 0:
                    nc.vector.tensor_scalar_mul(mapped, pmm, onehot[:, 0:1])
                else:
                    nc.vector.scalar_tensor_tensor(
                        out=mapped, in0=pmm, scalar=onehot[:, c:c + 1], in1=mapped,
                        op0=ALU.mult, op1=ALU.add)
            nc.vector.tensor_add(mapped, mapped, bwsl)

            # ---------- AllToAll: D-slices -> per-core full rows ----------
            ag_in = dram.tile([B, DSL], f32, tag="ag_in")
            ag_out = dram.tile([B, DSL], f32, tag="ag_out")
            nc.sync.dma_start(ag_in, mapped)
            if collective:
                nc.gpsimd.collective_compute(
                    "AllToAll", ALU.bypass,
                    replica_groups=[list(range(NCORES))],
                    ins=[ag_in[:, :].opt()], outs=[ag_out[:, :].opt()])
            else:
                # single-core timing variant: plain copy stands in for AllToAll
                nc.sync.dma_start(ag_out, ag_in[:, :])

            # ag_out[j*8 + p, :] = D-slice j of sample (8*core + p).
            # fold sample p's full row into [128, 8]: partition q holds D[8q..8q+8)
            mraw = state.tile([128, SAMP, FOLD], f32, tag="mraw")
            agv = ag_out[:, :].rearrange("(j p) (q c) -> j p q c", p=SAMP, c=FOLD)
            for p in range(SAMP):
                nc.sync.dma_start(mraw[:, p, :], agv[:, p, :, :])

            # ---------- loga = mraw - lse_D(mraw) per sample ----------
            def cross_partition_chain(src, nprob, op_alu):
                pr = scratch.tile([128, nprob], f32, tag="cp_pr")
                nc.vector.tensor_reduce(pr, src, axis=AX.X, op=op_alu)
                tp = psum_tr.tile([nprob, 128], f32, tag="tr")
                nc.tensor.transpose(tp, pr, ident)
                red = scratch.tile([nprob, 1], f32, tag="cp_red")
                nc.vector.tensor_reduce(red, tp, axis=AX.X, op=op_alu)
                return red

            def bcast_over_parts(col, nprob):
                tpc = psum_tr.tile([1, nprob], f32, tag="tr")
                nc.tensor.transpose(tpc, col, ident[0:nprob, 0:nprob])
                row = scratch.tile([1, nprob], f32, tag="bc_row")
                nc.scalar.copy(row, tpc)
                bc = psum_tr.tile([128, nprob], f32, tag="tr")
                nc.tensor.matmul(bc, lhsT=ones1, rhs=row, start=True, stop=True)
                return bc

            # mapped ~ N(0,1)-scale (|x| < ~10), so exp needs no max-stabilizer:
            # Za = ln(sum exp(mapped)) directly.
            e_e = scratch.tile([128, SAMP, FOLD], f32, tag="e_e")
            nc.scalar.activation(e_e, mraw, AF.Exp)
            ss_col = cross_partition_chain(e_e, SAMP, ALU.add)
            za = scratch.tile([SAMP, 1], f32, tag="za")
            nc.scalar.activation(za, ss_col, AF.Ln)
            bz = bcast_over_parts(za, SAMP)
            loga = state.tile([128, SAMP, FOLD], f32, tag="loga")
            nc.vector.tensor_sub(loga, mraw, bz[:, :, None].broadcast_to([128, SAMP, FOLD]))

            # ---------- marginals per group ----------
            # group A (9 probs): 0..7 ab (LA=loga_p, LB=logb_p), 8 bb (LA=LB=logb_cls)
            # group B (8 probs): aa (LA=LB=loga_p)
            NP = {0: NA, 1: NB}
            LA = {}; LB = {}
            LA[0] = state.tile([128, NA, FOLD], f32, tag="LA_A", name="LA_A")
            LB[0] = state.tile([128, NA, FOLD], f32, tag="LB_A", name="LB_A")
            LA[1] = state.tile([128, NB, FOLD], f32, tag="LA_B", name="LA_B")
            LB[1] = state.tile([128, NB, FOLD], f32, tag="LB_B", name="LB_B")
            nc.scalar.copy(LA[0][:, 0:SAMP, :], loga)
            nc.scalar.copy(LA[1][:, :, :], loga)
            nc.scalar.copy(LB[1][:, :, :], loga)
            for p in range(SAMP):
                nc.sync.dma_start(
                    LB[0][:, p, :],
                    logbs_d[p, :].rearrange("(q c) -> q c", c=FOLD))
            nc.sync.dma_start(
                LA[0][:, SAMP, :],
                logbs_d[SAMP, :].rearrange("(q c) -> q c", c=FOLD))
            nc.sync.dma_start(
                LB[0][:, SAMP, :],
                logbs_d[SAMP, :].rearrange("(q c) -> q c", c=FOLD))

            # ---------- Sinkhorn state (per group) ----------
            SFh = {}; SGh = {}; Mf = {}; Mg = {}; Tb = {}; Eb = {}
            acc = {}; lnacc = {}
            for g in (0, 1):
                s = "AB"[g]
                SFh[g] = state.tile([128, NP[g], HW], f32, tag=f"SFh{s}", name=f"SFh{s}")
                SGh[g] = state.tile([128, NP[g], HW], f32, tag=f"SGh{s}", name=f"SGh{s}")
                Mf[g] = state.tile([128, NP[g], FOLD], f32, tag=f"Mf{s}", name=f"Mf{s}")
                Mg[g] = state.tile([128, NP[g], FOLD], f32, tag=f"Mg{s}", name=f"Mg{s}")
                # band-slot order: [0]=d0, [1]=-1, [2]=+1, [3]=-2, [4]=+2, [5]=-3, [6]=+3
                Tb[g] = state.tile([128, NP[g], FOLD, 2 * R + 1], f32, tag=f"Tb{s}", name=f"Tb{s}")
                Eb[g] = state.tile([128, NP[g], FOLD, 2 * R + 1], f32, tag=f"Eb{s}", name=f"Eb{s}")
                acc[g] = state.tile([128, NP[g], FOLD], f32, tag=f"acc{s}", name=f"acc{s}")
                lnacc[g] = state.tile([128, NP[g], FOLD], f32, tag=f"lnacc{s}", name=f"lnacc{s}")
                nc.vector.memset(Mf[g], 0.0)
                nc.vector.memset(Mg[g], 0.0)
                nc.vector.memset(SFh[g], 0.0)
                nc.vector.memset(SGh[g], 0.0)

            def halo_fill(buf, g):
                np_ = NP[g]
                # both halos via one PSUM tile + one 2-range DVE add.
                # left halo: buf[q, :, 0:R] = buf[q-1, :, FOLD:FOLD+R]
                # right halo: buf[q, :, R+FOLD:HW] = buf[q+1, :, R:2R]
                # boundary partitions forced to NEG by the hmask tile.
                if HALO_DVE:
                    ph = psum_h.tile([128, 2, NA, R], f32, tag=f"halo{'AB'[g]}")
                    nc.tensor.matmul(ph[:, 0, 0:np_, :], lhsT=sdn,
                                     rhs=buf[:, :, FOLD:FOLD + R], start=True, stop=True)
                    nc.tensor.matmul(ph[:, 1, 0:np_, :], lhsT=sup,
                                     rhs=buf[:, :, R:2 * R], start=True, stop=True)
                    # one DVE add writes both halo col-ranges: dims (side, prob, col)
                    bb = buf[:, :, 0:R]
                    halo_out = bass.AP(tensor=bb.tensor, offset=bb.offset,
                                       ap=[bb.ap[0], [R + FOLD, 2], bb.ap[1], [1, R]])
                    ph_v = ph[:, :, 0:np_, :]
                    hs = hmask[:, :]
                    hm_v = bass.AP(tensor=hs.tensor, offset=hs.offset,
                                   ap=[hs.ap[0], [R, 2], [0, np_], [1, R]])
                    nc.vector.tensor_add(halo_out, ph_v, hm_v)
                else:
                    pl = psum_h.tile([128, NA, R], f32, tag=f"hl{'AB'[g]}")
                    nc.tensor.matmul(pl[:, 0:np_, :], lhsT=sdn,
                                     rhs=buf[:, :, FOLD:FOLD + R], start=True, stop=True)
                    nc.scalar.add(buf[:, :, 0:R], pl[:, 0:np_, :], bneg0[:, 0:1])
                    pr_ = psum_h.tile([128, NA, R], f32, tag=f"hr{'AB'[g]}")
                    nc.tensor.matmul(pr_[:, 0:np_, :], lhsT=sup,
                                     rhs=buf[:, :, R:2 * R], start=True, stop=True)
                    nc.scalar.add(buf[:, :, R + FOLD:HW], pr_[:, 0:np_, :],
                                  bneg127[:, 0:1])

            for g in (0, 1):
                nc.scalar.copy(SGh[g][:, :, R:R + FOLD], LB[g])
                nc.scalar.copy(SFh[g][:, :, R:R + FOLD], LA[g])
                halo_fill(SGh[g], g)

            def pair_ap(buf, k, np_):
                """[128, np, FOLD, 2] view of buf: pair (-k, +k) windows."""
                base = buf[:, :, R - k:R - k + FOLD]
                return bass.AP(tensor=base.tensor, offset=base.offset,
                               ap=[*base.ap, [2 * k, 2]])

            def bcast2_ap(t):
                """append a step-0 pair dim to a [128, np, FOLD] view."""
                return bass.AP(tensor=t.tensor, offset=t.offset,
                               ap=[*t.ap, [0, 2]])

            def half_update(g, src, dst, M, LAd):
                T, E = Tb[g], Eb[g]
                np_ = NP[g]
                # pair windows via 4D tensor_sub (walrus-legal); c_d goes into
                # the per-pair Exp bias (c identical for +-d).
                nc.vector.tensor_sub(T[:, :, :, 0], src[:, :, R:R + FOLD], M)
                for k in (1, 2, 3):
                    nc.vector.tensor_sub(
                        T[:, :, :, 2 * k - 1:2 * k + 1], pair_ap(src, k, np_),
                        bcast2_ap(M[:, :, :]))
                nc.scalar.activation(E[:, :, :, 0], T[:, :, :, 0], AF.Exp)
                for k in (1, 2, 3):
                    nc.scalar.activation(
                        E[:, :, :, 2 * k - 1:2 * k + 1], T[:, :, :, 2 * k - 1:2 * k + 1],
                        AF.Exp, bias=cbias[-20.0 * k * k][:, 0:1])
                nc.vector.tensor_reduce(acc[g], E, axis=AX.X, op=ALU.add)
                nc.scalar.activation(lnacc[g], acc[g], AF.Ln)
                # dst center: LA - M_new == dst_prev - lnacc (dst was LA - M_old);
                # keeps the halo shift off the M-update dependency
                nc.vector.tensor_sub(dst[:, :, R:R + FOLD],
                                     dst[:, :, R:R + FOLD], lnacc[g])
                nc.gpsimd.tensor_add(M, M, lnacc[g])
                halo_fill(dst, g)

            for _t in range(iters):
                half_update(0, SGh[0], SFh[0], Mf[0], LA[0])
                half_update(1, SGh[1], SFh[1], Mf[1], LA[1])
                half_update(0, SFh[0], SGh[0], Mg[0], LB[0])
                half_update(1, SFh[1], SGh[1], Mg[1], LB[1])

            # ---------- value: V = sum_{i,d} exp(SF_i+SG_{i+d}+c_d)(F_i+G_{i+d}) ----------
            vrows = {}
            for g in (0, 1):
                np_ = NP[g]
                Ft = scratch.tile([128, NA, FOLD], f32, tag="Ft", name="Ft")[:, 0:np_, :]
                nc.vector.tensor_sub(Ft, SFh[g][:, :, R:R + FOLD], LA[g])
                nc.vector.tensor_scalar_mul(Ft, Ft, float(EPS))
                Gh = scratch.tile([128, NA, HW], f32, tag="Gh", name="Gh")[:, 0:np_, :]
                nc.vector.tensor_sub(Gh[:, :, R:R + FOLD], SGh[g][:, :, R:R + FOLD], LB[g])
                nc.vector.tensor_scalar_mul(
                    Gh[:, :, R:R + FOLD], Gh[:, :, R:R + FOLD], float(EPS))
                halo_fill(Gh, g)

                s1 = scratch.tile([128, NA, FOLD], f32, tag="s1", name="s1")[:, 0:np_, :]
                pd_ = scratch.tile([128, NA, FOLD], f32, tag="pd", name="pd")[:, 0:np_, :]
                s2 = scratch.tile([128, NA, FOLD], f32, tag="s2", name="s2")[:, 0:np_, :]
                vacc = scratch.tile([128, NA, FOLD], f32, tag="vacc", name="vacc")[:, 0:np_, :]
                for di, d in enumerate(range(-R, R + 1)):
                    nc.vector.tensor_add(
                        s1, SFh[g][:, :, R:R + FOLD], SGh[g][:, :, R + d:R + d + FOLD])
                    nc.scalar.activation(pd_, s1, AF.Exp,
                                         bias=cbias[-20.0 * d * d][:, 0:1])
                    nc.vector.tensor_add(s2, Ft, Gh[:, :, R + d:R + d + FOLD])
                    if di == 0:
                        nc.vector.tensor_mul(vacc, pd_, s2)
                    else:
                        nc.vector.tensor_mul(s2, pd_, s2)
                        nc.vector.tensor_add(vacc, vacc, s2)

                vsum = scratch.tile([128, NA], f32, tag="vsum", name="vsum")[:, 0:np_]
                nc.vector.tensor_reduce(vsum, vacc, axis=AX.X, op=ALU.add)
                tv = psum_tr.tile([NA, 128], f32, tag="tr")
                nc.tensor.transpose(tv[0:np_, :], vsum, ident)
                vrow = scratch.tile([NA, 1], f32, tag=f"vrow{'AB'[g]}", name=f"vrow{'AB'[g]}")
                nc.vector.tensor_reduce(vrow[0:np_, :], tv[0:np_, :], axis=AX.X,
                                        op=ALU.add)
                vrows[g] = vrow

            # output layout: [0:8] ab, [8] bb, [9:17] aa
            nc.sync.dma_start(vals_d[0:NA], vrows[0][0:NA, :])
            nc.sync.dma_start(vals_d[NA:NPROB], vrows[1][0:NB, :])

    if not nc.is_finalized():
        # All ScalarE funcs used here (Exp, Ln, Identity, Copy) live in the
        # single act-table set "natural_log_exp_and_others"; hide the other
        # sets (preserving list positions = act_func_set_id) so the table
        # load pass emits one load instead of thrashing between
        # exp_and_others and natural_log every half-update.
        import concourse.bacc as bacc_mod
        _orig_gat = bacc_mod.get_activation_tables
        def _one_set_gat(arch):
            t = _orig_gat(arch)
            return {name: (fns if name == "natural_log_exp_and_others" else set())
                    for name, fns in t.items()}
        bacc_mod.get_activation_tables = _one_set_gat
        try:
            nc.finalize()
        finally:
            bacc_mod.get_activation_tables = _orig_gat
    return nc


def _log_softmax(x):
    m = x.max(axis=-1, keepdims=True)
    e = np.exp(x - m)
    return (x - m) - np.log(e.sum(axis=-1, keepdims=True))


def prep_in_maps(fea, y, W, bW, anchor_tab):
    fea = np.ascontiguousarray(np.asarray(fea, dtype=np.float32))
    y = np.asarray(y).astype(np.int64)
    W = np.asarray(W, dtype=np.float32)
    bW = np.asarray(bW, dtype=np.float32)
    anchor_tab = np.asarray(anchor_tab, dtype=np.float32)

    logb_cls = _log_softmax(anchor_tab).astype(np.float32)      # [5, D]
    bW_g = bW[y]                                                # [B, D]
    onehot = np.zeros((B, NUM_CLASSES), dtype=np.float32)
    onehot[np.arange(B), y] = 1.0

    in_maps = []
    for k in range(NCORES):
        sl = slice(k * DSL, (k + 1) * DSL)
        logbs = np.empty((SAMP + 1, D), dtype=np.float32)
        for p in range(SAMP):
            logbs[p] = logb_cls[y[k * SAMP + p]]
        logbs[SAMP] = logb_cls[k] if k < NUM_CLASSES else logb_cls[0]
        in_maps.append({
            "fea": fea,
            "wsl": np.ascontiguousarray(W[:, sl, :].transpose(0, 2, 1)),
            "bwsl": np.ascontiguousarray(bW_g[:, sl]),
            "onehot": onehot,
            "logbs": logbs,
        })
    return in_maps


def postprocess(vals_per_core, y):
    y = np.asarray(y).astype(np.int64)
    V_ab = np.empty(B, dtype=np.float32)
    V_aa = np.empty(B, dtype=np.float32)
    V_bb = np.empty(NUM_CLASSES, dtype=np.float32)
    for k in range(NCORES):
        v = np.asarray(vals_per_core[k]).reshape(-1)
        V_ab[k * SAMP:(k + 1) * SAMP] = v[0:SAMP]
        V_aa[k * SAMP:(k + 1) * SAMP] = v[NA:NPROB]
        if k < NUM_CLASSES:
            V_bb[k] = v[SAMP]
    losses = (2.0 * V_ab - V_aa - V_bb[y]).astype(np.float32)
    z = -losses * np.float32(TEMP)
    zm = z.max()
    e = np.exp(z - zm)
    weights = (e / e.sum()).astype(np.float32)
    loss = np.float32(np.sum(losses * weights))
    return loss, weights


_CACHE = {}


def kernel(fea, y, W, bW, anchor_tab):
    from concourse.bass_utils import run_bass_kernel_spmd

    if "nc" not in _CACHE:
        _CACHE["nc"] = _build_program(ITERS)
    nc = _CACHE["nc"]

    in_maps = prep_in_maps(fea, y, W, bW, anchor_tab)
    res = run_bass_kernel_spmd(nc, in_maps, list(range(NCORES)))
    vals = [res.results[k]["values"] for k in range(NCORES)]
    return postprocess(vals, y)


# revision 37
# speedup vs baseline: 1.0384x; 1.0006x over previous
"""Trainium2 Bass kernel for the Sinkhorn-OT loss problem.

Math summary (mirrors the reference):
  mapped = einsum('bf,bdf->bd', fea, W[y]) + bW[y];  a = softmax(mapped)
  b = softmax(anchor_tab[y]);  M_ij = (i-j)^2 on D=1024, eps=0.05
  ot(x, y) = 50-iteration log-domain Sinkhorn value
  losses = 2*ot(a,b) - ot(a,a) - ot(b,b);  weights = softmax(-losses)
  loss = sum(losses*weights)

Key structural facts exploited:
  * The Gibbs kernel exp(-(i-j)^2/eps) decays by e^-20 per unit distance, so
    every logsumexp over D is exactly banded with radius 3 in f32 (terms at
    |d|=4 sit >= 40 below the row max - invisible at f32 precision).
  * ot(b,b) has only 5 distinct instances (one per class).
  * The warm-start stabilizer (previous iteration's lse output) keeps all
    exp arguments in [-inf, ~7]; no per-row max pass is needed.

Distribution over the 8 cores:
  Phase A: D-parallel matmul. Core k holds W[:, 128k:128k+128, :], computes
    that D-slice of mapped for all 64 samples on the TensorEngine, then an
    AllToAll redistributes slices so core k owns full rows for samples
    8k..8k+8.
  Phase B: problem-parallel Sinkhorn. Each core runs 17 independent solves
    in two interleaved groups (A: 8 ab + 1 bb, B: 8 aa) so the two serial
    update chains fill each other's pipeline gaps. Each problem's D=1024
    axis is folded as [128 partitions x 8 cols] plus a 3-wide halo
    maintained by TensorEngine shift-matmuls.
  Host: assembles the 133 values, forms losses/weights/loss (O(64) work).

Problem order in the "values" output: [0:8] ab, [8] bb, [9:17] aa.
"""

import numpy as np

NUM_CLASSES = 5
FEAT = 2048
D = 1024
B = 64
EPS = 0.05
ITERS = 50
TEMP = 1.0

R = 3                 # band radius
HW = 8 + 2 * R        # halo'd fold width (14)
NCORES = 8
SAMP = B // NCORES    # samples per core (8)
NPROB = 2 * SAMP + 1  # problems per core (17)
DSL = D // NCORES     # D-slice per core (128)
FOLD = D // 128       # fold width (8)
NEG = -1.0e30
import os
EXP_SPLIT = int(os.environ.get('EXP_SPLIT', '1'))
HALO_DVE = int(os.environ.get('HALO_DVE', '0'))
NA = SAMP + 1         # group A: 8 ab + 1 bb
NB = SAMP             # group B: 8 aa


def _build_program(iters=ITERS, collective=True):
    import concourse.bass as bass
    import concourse.bacc as bacc
    import concourse.tile as tile
    from concourse import mybir

    f32 = mybir.dt.float32
    AF = mybir.ActivationFunctionType
    ALU = mybir.AluOpType
    AX = mybir.AxisListType

    nc = bacc.Bacc("TRN2", target_bir_lowering=False, debug=False)

    fea_d = nc.declare_dram_parameter("fea", [B, FEAT], f32, isOutput=False)
    wsl_d = nc.declare_dram_parameter("wsl", [NUM_CLASSES, FEAT, DSL], f32, isOutput=False)
    bwsl_d = nc.declare_dram_parameter("bwsl", [B, DSL], f32, isOutput=False)
    onehot_d = nc.declare_dram_parameter("onehot", [B, NUM_CLASSES], f32, isOutput=False)
    logbs_d = nc.declare_dram_parameter("logbs", [SAMP + 1, D], f32, isOutput=False)
    vals_d = nc.declare_dram_parameter("values", [NPROB], f32, isOutput=True)

    ident_d = nc.inline_tensor(np.eye(128, dtype=np.float32), name="ident")
    # shift-down: out[q] = in[q-1]  (SD[k, m] = 1 iff k = m-1)
    sd_d = nc.inline_tensor(np.eye(128, k=1, dtype=np.float32), name="sdn")
    # shift-up: out[q] = in[q+1]  (SU[k, m] = 1 iff k = m+1)
    su_d = nc.inline_tensor(np.eye(128, k=-1, dtype=np.float32), name="sup")
    ones_d = nc.inline_tensor(np.ones((1, 128), dtype=np.float32), name="ones1")
    bneg0_np = np.zeros((128, 1), dtype=np.float32); bneg0_np[0, 0] = NEG
    bneg127_np = np.zeros((128, 1), dtype=np.float32); bneg127_np[127, 0] = NEG
    bneg0_d = nc.inline_tensor(bneg0_np, name="bneg0")
    bneg127_d = nc.inline_tensor(bneg127_np, name="bneg127")
    # halo boundary mask: cols 0:3 NEG at partition 0, cols 3:6 NEG at partition 127
    hm_np = np.zeros((128, 6), dtype=np.float32)
    hm_np[0, 0:3] = NEG; hm_np[127, 3:6] = NEG
    hmask_d = nc.inline_tensor(hm_np, name="hmask")

    with tile.TileContext(nc) as tc:
        with (
            tc.tile_pool(name="consts", bufs=1) as consts,
            tc.tile_pool(name="pha", bufs=2) as pha,
            tc.tile_pool(name="state", bufs=1) as state,
            tc.tile_pool(name="scratch", bufs=2) as scratch,
            tc.tile_pool(name="psum_mm", bufs=2, space="PSUM") as psum_mm,
            tc.tile_pool(name="psum_tr", bufs=2, space="PSUM") as psum_tr,
            tc.tile_pool(name="psum_h", bufs=1, space="PSUM") as psum_h,
            tc.tile_pool(name="dram", bufs=1, space="DRAM") as dram,
        ):
            # ---------- constants to SBUF ----------
            ident = consts.tile([128, 128], f32, tag="ident")
            nc.sync.dma_start(ident, ident_d[:, :])
            sdn = consts.tile([128, 128], f32, tag="sdn")
            nc.sync.dma_start(sdn, sd_d[:, :])
            sup = consts.tile([128, 128], f32, tag="sup")
            nc.sync.dma_start(sup, su_d[:, :])
            ones1 = consts.tile([1, 128], f32, tag="ones1")
            nc.sync.dma_start(ones1, ones_d[:, :])
            onehot = consts.tile([B, NUM_CLASSES], f32, tag="onehot")
            nc.sync.dma_start(onehot, onehot_d[:, :])
            bwsl = consts.tile([B, DSL], f32, tag="bwsl")
            nc.sync.dma_start(bwsl, bwsl_d[:, :])
            bneg0 = consts.tile([128, 1], f32, tag="bneg0")
            nc.sync.dma_start(bneg0, bneg0_d[:, :])
            bneg127 = consts.tile([128, 1], f32, tag="bneg127")
            nc.sync.dma_start(bneg127, bneg127_d[:, :])
            hmask = consts.tile([128, 6], f32, tag="hmask")
            nc.sync.dma_start(hmask, hmask_d[:, :])

            # per-partition bias constants for activation (c_d = -20*d^2)
            cbias = {}
            for v in sorted({-20.0 * d * d for d in range(0, R + 1)}):
                t = consts.tile([128, 1], f32, tag=f"cb{int(-v)}")
                nc.vector.memset(t, v)
                cbias[v] = t

            # ---------- Phase A: mapped D-slice for all 64 samples ----------
            fea_sb = consts.tile([B, FEAT], f32, tag="fea")
            nc.sync.dma_start(fea_sb, fea_d[:, :])

            feaT = consts.tile([128, 16, B], f32, tag="feaT")
            for j in range(16):
                pt = psum_tr.tile([128, B], f32, tag="tr")
                nc.tensor.transpose(pt, fea_sb[:, j * 128:(j + 1) * 128], ident[0:B, 0:B])
                if j % 2 == 0:
                    nc.vector.tensor_copy(feaT[:, j, :], pt)
                else:
                    nc.scalar.copy(feaT[:, j, :], pt)

            mapped = state.tile([B, DSL], f32, tag="mapped")
            for c in range(NUM_CLASSES):
                # W arrives host-pre-transposed as [C, FEAT, DSL]: DMA the
                # [feat-chunk, d] tiles straight into matmul-ready layout.
                wcT = pha.tile([128, 16, DSL], f32, tag="wcT")
                wv = wsl_d[c, :, :].rearrange("(j p) d -> p j d", p=128)
                for h in range(4):
                    nc.sync.dma_start(wcT[:, h * 4:(h + 1) * 4, :],
                                      wv[:, h * 4:(h + 1) * 4, :])
                pmm = psum_mm.tile([B, DSL], f32, tag="pmm")
                for j in range(16):
                    nc.tensor.matmul(pmm, lhsT=feaT[:, j, :], rhs=wcT[:, j, :],
                                     start=(j == 0), stop=(j == 15))
                if c == 0:
                    nc.vector.tensor_scalar_mul(mapped, pmm, onehot[:, 0:1])
                else:
                    nc.vector.scalar_tensor_tensor(
                        out=mapped, in0=pmm, scalar=onehot[:, c:c + 1], in1=mapped,
                        op0=ALU.mult, op1=ALU.add)
            nc.vector.tensor_add(mapped, mapped, bwsl)

            # ---------- AllToAll: D-slices -> per-core full rows ----------
            ag_in = dram.tile([B, DSL], f32, tag="ag_in")
            ag_out = dram.tile([B, DSL], f32, tag="ag_out")
            nc.sync.dma_start(ag_in, mapped)
            if collective:
                nc.gpsimd.collective_compute(
                    "AllToAll", ALU.bypass,
                    replica_groups=[list(range(NCORES))],
                    ins=[ag_in[:, :].opt()], outs=[ag_out[:, :].opt()])
            else:
                # single-core timing variant: plain copy stands in for AllToAll
                nc.sync.dma_start(ag_out, ag_in[:, :])

            # ag_out[j*8 + p, :] = D-slice j of sample (8*core + p).
            # fold sample p's full row into [128, 8]: partition q holds D[8q..8q+8)
            mraw = state.tile([128, SAMP, FOLD], f32, tag="mraw")
            agv = ag_out[:, :].rearrange("(j p) (q c) -> j p q c", p=SAMP, c=FOLD)
            for p in range(SAMP):
                nc.sync.dma_start(mraw[:, p, :], agv[:, p, :, :])

            # ---------- loga = mraw - lse_D(mraw) per sample ----------
            def cross_partition_chain(src, nprob, op_alu):
                pr = scratch.tile([128, nprob], f32, tag="cp_pr")
                nc.vector.tensor_reduce(pr, src, axis=AX.X, op=op_alu)
                tp = psum_tr.tile([nprob, 128], f32, tag="tr")
                nc.tensor.transpose(tp, pr, ident)
                red = scratch.tile([nprob, 1], f32, tag="cp_red")
                nc.vector.tensor_reduce(red, tp, axis=AX.X, op=op_alu)
                return red

            def bcast_over_parts(col, nprob):
                tpc = psum_tr.tile([1, nprob], f32, tag="tr")
                nc.tensor.transpose(tpc, col, ident[0:nprob, 0:nprob])
                row = scratch.tile([1, nprob], f32, tag="bc_row")
                nc.scalar.copy(row, tpc)
                bc = psum_tr.tile([128, nprob], f32, tag="tr")
                nc.tensor.matmul(bc, lhsT=ones1, rhs=row, start=True, stop=True)
                return bc

            # mapped ~ N(0,1)-scale (|x| < ~10), so exp needs no max-stabilizer:
            # Za = ln(sum exp(mapped)) directly.
            e_e = scratch.tile([128, SAMP, FOLD], f32, tag="e_e")
            nc.scalar.activation(e_e, mraw, AF.Exp)
            ss_col = cross_partition_chain(e_e, SAMP, ALU.add)
            za = scratch.tile([SAMP, 1], f32, tag="za")
            nc.scalar.activation(za, ss_col, AF.Ln)
            bz = bcast_over_parts(za, SAMP)
            loga = state.tile([128, SAMP, FOLD], f32, tag="loga")
            nc.vector.tensor_sub(loga, mraw, bz[:, :, None].broadcast_to([128, SAMP, FOLD]))

            # ---------- marginals per group ----------
            # group A (9 probs): 0..7 ab (LA=loga_p, LB=logb_p), 8 bb (LA=LB=logb_cls)
            # group B (8 probs): aa (LA=LB=loga_p)
            NP = {0: NA, 1: NB}
            LA = {}; LB = {}
            LA[0] = state.tile([128, NA, FOLD], f32, tag="LA_A", name="LA_A")
            LB[0] = state.tile([128, NA, FOLD], f32, tag="LB_A", name="LB_A")
            LA[1] = state.tile([128, NB, FOLD], f32, tag="LA_B", name="LA_B")
            LB[1] = state.tile([128, NB, FOLD], f32, tag="LB_B", name="LB_B")
            nc.scalar.copy(LA[0][:, 0:SAMP, :], loga)
            nc.scalar.copy(LA[1][:, :, :], loga)
            nc.scalar.copy(LB[1][:, :, :], loga)
            for p in range(SAMP):
                nc.sync.dma_start(
                    LB[0][:, p, :],
                    logbs_d[p, :].rearrange("(q c) -> q c", c=FOLD))
            nc.sync.dma_start(
                LA[0][:, SAMP, :],
                logbs_d[SAMP, :].rearrange("(q c) -> q c", c=FOLD))
            nc.sync.dma_start(
                LB[0][:, SAMP, :],
                logbs_d[SAMP, :].rearrange("(q c) -> q c", c=FOLD))

            # ---------- Sinkhorn state (per group) ----------
            SFh = {}; SGh = {}; Mf = {}; Mg = {}; Tb = {}; Eb = {}
            acc = {}; lnacc = {}
            for g in (0, 1):
                s = "AB"[g]
                SFh[g] = state.tile([128, NP[g], HW], f32, tag=f"SFh{s}", name=f"SFh{s}")
                SGh[g] = state.tile([128, NP[g], HW], f32, tag=f"SGh{s}", name=f"SGh{s}")
                Mf[g] = state.tile([128, NP[g], FOLD], f32, tag=f"Mf{s}", name=f"Mf{s}")
                Mg[g] = state.tile([128, NP[g], FOLD], f32, tag=f"Mg{s}", name=f"Mg{s}")
                # band-slot order: [0]=d0, [1]=-1, [2]=+1, [3]=-2, [4]=+2, [5]=-3, [6]=+3
                Tb[g] = state.tile([128, NP[g], FOLD, 2 * R + 1], f32, tag=f"Tb{s}", name=f"Tb{s}")
                Eb[g] = state.tile([128, NP[g], FOLD, 2 * R + 1], f32, tag=f"Eb{s}", name=f"Eb{s}")
                acc[g] = state.tile([128, NP[g], FOLD], f32, tag=f"acc{s}", name=f"acc{s}")
                lnacc[g] = state.tile([128, NP[g], FOLD], f32, tag=f"lnacc{s}", name=f"lnacc{s}")
                nc.vector.memset(Mf[g], 0.0)
                nc.vector.memset(Mg[g], 0.0)
                nc.vector.memset(SFh[g], 0.0)
                nc.vector.memset(SGh[g], 0.0)

            def halo_fill(buf, g):
                np_ = NP[g]
                # both halos via one PSUM tile + one 2-range DVE add.
                # left halo: buf[q, :, 0:R] = buf[q-1, :, FOLD:FOLD+R]
                # right halo: buf[q, :, R+FOLD:HW] = buf[q+1, :, R:2R]
                # boundary partitions forced to NEG by the hmask tile.
                if HALO_DVE:
                    ph = psum_h.tile([128, 2, NA, R], f32, tag=f"halo{'AB'[g]}")
                    nc.tensor.matmul(ph[:, 0, 0:np_, :], lhsT=sdn,
                                     rhs=buf[:, :, FOLD:FOLD + R], start=True, stop=True)
                    nc.tensor.matmul(ph[:, 1, 0:np_, :], lhsT=sup,
                                     rhs=buf[:, :, R:2 * R], start=True, stop=True)
                    # one DVE add writes both halo col-ranges: dims (side, prob, col)
                    bb = buf[:, :, 0:R]
                    halo_out = bass.AP(tensor=bb.tensor, offset=bb.offset,
                                       ap=[bb.ap[0], [R + FOLD, 2], bb.ap[1], [1, R]])
                    ph_v = ph[:, :, 0:np_, :]
                    hs = hmask[:, :]
                    hm_v = bass.AP(tensor=hs.tensor, offset=hs.offset,
                                   ap=[hs.ap[0], [R, 2], [0, np_], [1, R]])
                    nc.vector.tensor_add(halo_out, ph_v, hm_v)
                else:
                    pl = psum_h.tile([128, NA, R], f32, tag=f"hl{'AB'[g]}")
                    nc.tensor.matmul(pl[:, 0:np_, :], lhsT=sdn,
                                     rhs=buf[:, :, FOLD:FOLD + R], start=True, stop=True)
                    nc.scalar.add(buf[:, :, 0:R], pl[:, 0:np_, :], bneg0[:, 0:1])
                    pr_ = psum_h.tile([128, NA, R], f32, tag=f"hr{'AB'[g]}")
                    nc.tensor.matmul(pr_[:, 0:np_, :], lhsT=sup,
                                     rhs=buf[:, :, R:2 * R], start=True, stop=True)
                    nc.scalar.add(buf[:, :, R + FOLD:HW], pr_[:, 0:np_, :],
                                  bneg127[:, 0:1])

            for g in (0, 1):
                nc.scalar.copy(SGh[g][:, :, R:R + FOLD], LB[g])
                nc.scalar.copy(SFh[g][:, :, R:R + FOLD], LA[g])
                halo_fill(SGh[g], g)

            def pair_ap(buf, k, np_):
                """[128, np, FOLD, 2] view of buf: pair (-k, +k) windows."""
                base = buf[:, :, R - k:R - k + FOLD]
                return bass.AP(tensor=base.tensor, offset=base.offset,
                               ap=[*base.ap, [2 * k, 2]])

            def bcast2_ap(t):
                """append a step-0 pair dim to a [128, np, FOLD] view."""
                return bass.AP(tensor=t.tensor, offset=t.offset,
                               ap=[*t.ap, [0, 2]])

            def half_update(g, src, dst, M, LAd):
                T, E = Tb[g], Eb[g]
                np_ = NP[g]
                # pair windows via 4D tensor_sub (walrus-legal); c_d goes into
                # the per-pair Exp bias (c identical for +-d).
                nc.vector.tensor_sub(T[:, :, :, 0], src[:, :, R:R + FOLD], M)
                for k in (1, 2, 3):
                    nc.vector.tensor_sub(
                        T[:, :, :, 2 * k - 1:2 * k + 1], pair_ap(src, k, np_),
                        bcast2_ap(M[:, :, :]))
                nc.scalar.activation(E[:, :, :, 0], T[:, :, :, 0], AF.Exp)
                for k in (1, 2, 3):
                    nc.scalar.activation(
                        E[:, :, :, 2 * k - 1:2 * k + 1], T[:, :, :, 2 * k - 1:2 * k + 1],
                        AF.Exp, bias=cbias[-20.0 * k * k][:, 0:1])
                nc.vector.tensor_reduce(acc[g], E, axis=AX.X, op=ALU.add)
                nc.scalar.activation(lnacc[g], acc[g], AF.Ln)
                # dst center: LA - M_new == dst_prev - lnacc (dst was LA - M_old);
                # keeps the halo shift off the M-update dependency
                nc.vector.tensor_sub(dst[:, :, R:R + FOLD],
                                     dst[:, :, R:R + FOLD], lnacc[g])
                nc.gpsimd.tensor_add(M, M, lnacc[g])
                halo_fill(dst, g)

            for _t in range(iters):
                half_update(0, SGh[0], SFh[0], Mf[0], LA[0])
                half_update(1, SGh[1], SFh[1], Mf[1], LA[1])
                half_update(0, SFh[0], SGh[0], Mg[0], LB[0])
                half_update(1, SFh[1], SGh[1], Mg[1], LB[1])

            # ---------- value: V = sum_{i,d} exp(SF_i+SG_{i+d}+c_d)(F_i+G_{i+d}) ----------
            vrows = {}
            for g in (0, 1):
                np_ = NP[g]
                Ft = scratch.tile([128, NA, FOLD], f32, tag="Ft", name="Ft")[:, 0:np_, :]
                nc.vector.tensor_sub(Ft, SFh[g][:, :, R:R + FOLD], LA[g])
                nc.vector.tensor_scalar_mul(Ft, Ft, float(EPS))
                Gh = scratch.tile([128, NA, HW], f32, tag="Gh", name="Gh")[:, 0:np_, :]
                nc.vector.tensor_sub(Gh[:, :, R:R + FOLD], SGh[g][:, :, R:R + FOLD], LB[g])
                nc.vector.tensor_scalar_mul(
                    Gh[:, :, R:R + FOLD], Gh[:, :, R:R + FOLD], float(EPS))
                halo_fill(Gh, g)

                s1 = scratch.tile([128, NA, FOLD], f32, tag="s1", name="s1")[:, 0:np_, :]
                pd_ = scratch.tile([128, NA, FOLD], f32, tag="pd", name="pd")[:, 0:np_, :]
                s2 = scratch.tile([128, NA, FOLD], f32, tag="s2", name="s2")[:, 0:np_, :]
                vacc = scratch.tile([128, NA, FOLD], f32, tag="vacc", name="vacc")[:, 0:np_, :]
                for di, d in enumerate(range(-R, R + 1)):
                    nc.vector.tensor_add(
                        s1, SFh[g][:, :, R:R + FOLD], SGh[g][:, :, R + d:R + d + FOLD])
                    nc.scalar.activation(pd_, s1, AF.Exp,
                                         bias=cbias[-20.0 * d * d][:, 0:1])
                    nc.vector.tensor_add(s2, Ft, Gh[:, :, R + d:R + d + FOLD])
                    if di == 0:
                        nc.vector.tensor_mul(vacc, pd_, s2)
                    else:
                        nc.vector.tensor_mul(s2, pd_, s2)
                        nc.vector.tensor_add(vacc, vacc, s2)

                vsum = scratch.tile([128, NA], f32, tag="vsum", name="vsum")[:, 0:np_]
                nc.vector.tensor_reduce(vsum, vacc, axis=AX.X, op=ALU.add)
                tv = psum_tr.tile([NA, 128], f32, tag="tr")
                nc.tensor.transpose(tv[0:np_, :], vsum, ident)
                vrow = scratch.tile([NA, 1], f32, tag=f"vrow{'AB'[g]}", name=f"vrow{'AB'[g]}")
                nc.vector.tensor_reduce(vrow[0:np_, :], tv[0:np_, :], axis=AX.X,
                                        op=ALU.add)
                vrows[g] = vrow

            # output layout: [0:8] ab, [8] bb, [9:17] aa
            nc.sync.dma_start(vals_d[0:NA], vrows[0][0:NA, :])
            nc.sync.dma_start(vals_d[NA:NPROB], vrows[1][0:NB, :])

    if not nc.is_finalized():
        # All ScalarE funcs used here (Exp, Ln, Identity, Copy) live in the
        # single act-table set "natural_log_exp_and_others"; hide the other
        # sets (preserving list positions = act_func_set_id) so the table
        # load pass emits one load instead of thrashing between
        # exp_and_others and natural_log every half-update.
        import concourse.bacc as bacc_mod
        _orig_gat = bacc_mod.get_activation_tables
        def _one_set_gat(arch):
            t = _orig_gat(arch)
            return {name: (fns if name == "natural_log_exp_and_others" else set())
                    for name, fns in t.items()}
        bacc_mod.get_activation_tables = _one_set_gat
        try:
            nc.finalize()
        finally:
            bacc_mod.get_activation_tables = _orig_gat
    return nc


def _log_softmax(x):
    m = x.max(axis=-1, keepdims=True)
    e = np.exp(x - m)
    return (x - m) - np.log(e.sum(axis=-1, keepdims=True))


def prep_in_maps(fea, y, W, bW, anchor_tab):
    fea = np.ascontiguousarray(np.asarray(fea, dtype=np.float32))
    y = np.asarray(y).astype(np.int64)
    W = np.asarray(W, dtype=np.float32)
    bW = np.asarray(bW, dtype=np.float32)
    anchor_tab = np.asarray(anchor_tab, dtype=np.float32)

    logb_cls = _log_softmax(anchor_tab).astype(np.float32)      # [5, D]
    bW_g = bW[y]                                                # [B, D]
    onehot = np.zeros((B, NUM_CLASSES), dtype=np.float32)
    onehot[np.arange(B), y] = 1.0

    in_maps = []
    for k in range(NCORES):
        sl = slice(k * DSL, (k + 1) * DSL)
        logbs = np.empty((SAMP + 1, D), dtype=np.float32)
        for p in range(SAMP):
            logbs[p] = logb_cls[y[k * SAMP + p]]
        logbs[SAMP] = logb_cls[k] if k < NUM_CLASSES else logb_cls[0]
        in_maps.append({
            "fea": fea,
            "wsl": np.ascontiguousarray(W[:, sl, :].transpose(0, 2, 1)),
            "bwsl": np.ascontiguousarray(bW_g[:, sl]),
            "onehot": onehot,
            "logbs": logbs,
        })
    return in_maps


def postprocess(vals_per_core, y):
    y = np.asarray(y).astype(np.int64)
    V_ab = np.empty(B, dtype=np.float32)
    V_aa = np.empty(B, dtype=np.float32)
    V_bb = np.empty(NUM_CLASSES, dtype=np.float32)
    for k in range(NCORES):
        v = np.asarray(vals_per_core[k]).reshape(-1)
        V_ab[k * SAMP:(k + 1) * SAMP] = v[0:SAMP]
        V_aa[k * SAMP:(k + 1) * SAMP] = v[NA:NPROB]
        if k < NUM_CLASSES:
            V_bb[k] = v[SAMP]
    losses = (2.0 * V_ab - V_aa - V_bb[y]).astype(np.float32)
    z = -losses * np.float32(TEMP)
    zm = z.max()
    e = np.exp(z - zm)
    weights = (e / e.sum()).astype(np.float32)
    loss = np.float32(np.sum(losses * weights))
    return loss, weights


_CACHE = {}


def kernel(fea, y, W, bW, anchor_tab):
    from concourse.bass_utils import run_bass_kernel_spmd

    if "nc" not in _CACHE:
        _CACHE["nc"] = _build_program(ITERS)
    nc = _CACHE["nc"]

    in_maps = prep_in_maps(fea, y, W, bW, anchor_tab)
    res = run_bass_kernel_spmd(nc, in_maps, list(range(NCORES)))
    vals = [res.results[k]["values"] for k in range(NCORES)]
    return postprocess(vals, y)


# revision 38
# speedup vs baseline: 1.1469x; 1.1044x over previous
"""Trainium2 Bass kernel for the Sinkhorn-OT loss problem.

Math summary (mirrors the reference):
  mapped = einsum('bf,bdf->bd', fea, W[y]) + bW[y];  a = softmax(mapped)
  b = softmax(anchor_tab[y]);  M_ij = (i-j)^2 on D=1024, eps=0.05
  ot(x, y) = 50-iteration log-domain Sinkhorn value
  losses = 2*ot(a,b) - ot(a,a) - ot(b,b);  weights = softmax(-losses)
  loss = sum(losses*weights)

Key structural facts exploited:
  * The Gibbs kernel exp(-(i-j)^2/eps) decays by e^-20 per unit distance, so
    every logsumexp over D is exactly banded with radius 3 in f32 (terms at
    |d|=4 sit >= 40 below the row max - invisible at f32 precision).
  * ot(b,b) has only 5 distinct instances (one per class).
  * The warm-start stabilizer (previous iteration's lse output) keeps all
    exp arguments in [-inf, ~7]; no per-row max pass is needed.

Distribution over the 8 cores:
  Phase A: D-parallel matmul. Core k holds W[:, 128k:128k+128, :], computes
    that D-slice of mapped for all 64 samples on the TensorEngine, then an
    AllToAll redistributes slices so core k owns full rows for samples
    8k..8k+8.
  Phase B: problem-parallel Sinkhorn. Each core runs 17 independent solves
    in two interleaved groups (A: 8 ab + 1 bb, B: 8 aa) so the two serial
    update chains fill each other's pipeline gaps. Each problem's D=1024
    axis is folded as [128 partitions x 8 cols] plus a 3-wide halo
    maintained by TensorEngine shift-matmuls.
  Host: assembles the 133 values, forms losses/weights/loss (O(64) work).

Problem order in the "values" output: [0:8] ab, [8] bb, [9:17] aa.
"""

import numpy as np

NUM_CLASSES = 5
FEAT = 2048
D = 1024
B = 64
EPS = 0.05
ITERS = 50
TEMP = 1.0

R = 3                 # band radius
HW = 8 + 2 * R        # halo'd fold width (14)
NCORES = 8
SAMP = B // NCORES    # samples per core (8)
NPROB = 2 * SAMP + 1  # problems per core (17)
DSL = D // NCORES     # D-slice per core (128)
FOLD = D // 128       # fold width (8)
NEG = -1.0e30
import os
EXP_SPLIT = int(os.environ.get('EXP_SPLIT', '1'))
HALO_DVE = int(os.environ.get('HALO_DVE', '0'))
NA = SAMP + 1         # group A: 8 ab + 1 bb
NB = SAMP             # group B: 8 aa


def _build_program(iters=ITERS, collective=True):
    import concourse.bass as bass
    import concourse.bacc as bacc
    import concourse.tile as tile
    from concourse import mybir

    f32 = mybir.dt.float32
    AF = mybir.ActivationFunctionType
    ALU = mybir.AluOpType
    AX = mybir.AxisListType

    nc = bacc.Bacc("TRN2", target_bir_lowering=False, debug=False)

    fea_d = nc.declare_dram_parameter("fea", [B, FEAT], f32, isOutput=False)
    wsl_d = nc.declare_dram_parameter("wsl", [NUM_CLASSES, FEAT, DSL], f32, isOutput=False)
    bwsl_d = nc.declare_dram_parameter("bwsl", [B, DSL], f32, isOutput=False)
    onehot_d = nc.declare_dram_parameter("onehot", [B, NUM_CLASSES], f32, isOutput=False)
    logbs_d = nc.declare_dram_parameter("logbs", [SAMP + 1, D], f32, isOutput=False)
    vals_d = nc.declare_dram_parameter("values", [NPROB], f32, isOutput=True)

    ident_d = nc.inline_tensor(np.eye(128, dtype=np.float32), name="ident")
    # shift-down: out[q] = in[q-1]  (SD[k, m] = 1 iff k = m-1)
    sd_d = nc.inline_tensor(np.eye(128, k=1, dtype=np.float32), name="sdn")
    # shift-up: out[q] = in[q+1]  (SU[k, m] = 1 iff k = m+1)
    su_d = nc.inline_tensor(np.eye(128, k=-1, dtype=np.float32), name="sup")
    ones_d = nc.inline_tensor(np.ones((1, 128), dtype=np.float32), name="ones1")
    bneg0_np = np.zeros((128, 1), dtype=np.float32); bneg0_np[0, 0] = NEG
    bneg127_np = np.zeros((128, 1), dtype=np.float32); bneg127_np[127, 0] = NEG
    bneg0_d = nc.inline_tensor(bneg0_np, name="bneg0")
    bneg127_d = nc.inline_tensor(bneg127_np, name="bneg127")
    # halo boundary mask: cols 0:3 NEG at partition 0, cols 3:6 NEG at partition 127
    hm_np = np.zeros((128, 6), dtype=np.float32)
    hm_np[0, 0:3] = NEG; hm_np[127, 3:6] = NEG
    hmask_d = nc.inline_tensor(hm_np, name="hmask")

    with tile.TileContext(nc) as tc:
        with (
            tc.tile_pool(name="consts", bufs=1) as consts,
            tc.tile_pool(name="pha", bufs=2) as pha,
            tc.tile_pool(name="state", bufs=1) as state,
            tc.tile_pool(name="scratch", bufs=2) as scratch,
            tc.tile_pool(name="psum_mm", bufs=2, space="PSUM") as psum_mm,
            tc.tile_pool(name="psum_tr", bufs=2, space="PSUM") as psum_tr,
            tc.tile_pool(name="psum_h", bufs=1, space="PSUM") as psum_h,
            tc.tile_pool(name="dram", bufs=1, space="DRAM") as dram,
        ):
            # ---------- constants to SBUF ----------
            ident = consts.tile([128, 128], f32, tag="ident")
            nc.sync.dma_start(ident, ident_d[:, :])
            sdn = consts.tile([128, 128], f32, tag="sdn")
            nc.sync.dma_start(sdn, sd_d[:, :])
            sup = consts.tile([128, 128], f32, tag="sup")
            nc.sync.dma_start(sup, su_d[:, :])
            ones1 = consts.tile([1, 128], f32, tag="ones1")
            nc.sync.dma_start(ones1, ones_d[:, :])
            onehot = consts.tile([B, NUM_CLASSES], f32, tag="onehot")
            nc.sync.dma_start(onehot, onehot_d[:, :])
            bwsl = consts.tile([B, DSL], f32, tag="bwsl")
            nc.sync.dma_start(bwsl, bwsl_d[:, :])
            bneg0 = consts.tile([128, 1], f32, tag="bneg0")
            nc.sync.dma_start(bneg0, bneg0_d[:, :])
            bneg127 = consts.tile([128, 1], f32, tag="bneg127")
            nc.sync.dma_start(bneg127, bneg127_d[:, :])
            hmask = consts.tile([128, 6], f32, tag="hmask")
            nc.sync.dma_start(hmask, hmask_d[:, :])

            # per-partition bias constants for activation (c_d = -20*d^2)
            cbias = {}
            for v in sorted({-20.0 * d * d for d in range(0, R + 1)}):
                t = consts.tile([128, 1], f32, tag=f"cb{int(-v)}")
                nc.vector.memset(t, v)
                cbias[v] = t

            # ---------- Phase A: mapped D-slice for all 64 samples ----------
            fea_sb = consts.tile([B, FEAT], f32, tag="fea")
            nc.sync.dma_start(fea_sb, fea_d[:, :])

            feaT = consts.tile([128, 16, B], f32, tag="feaT")
            for j in range(16):
                pt = psum_tr.tile([128, B], f32, tag="tr")
                nc.tensor.transpose(pt, fea_sb[:, j * 128:(j + 1) * 128], ident[0:B, 0:B])
                if j % 2 == 0:
                    nc.vector.tensor_copy(feaT[:, j, :], pt)
                else:
                    nc.scalar.copy(feaT[:, j, :], pt)

            mapped = state.tile([B, DSL], f32, tag="mapped")
            for c in range(NUM_CLASSES):
                # W arrives host-pre-transposed as [C, FEAT, DSL]: DMA the
                # [feat-chunk, d] tiles straight into matmul-ready layout.
                wcT = pha.tile([128, 16, DSL], f32, tag="wcT")
                wv = wsl_d[c, :, :].rearrange("(j p) d -> p j d", p=128)
                for h in range(4):
                    nc.sync.dma_start(wcT[:, h * 4:(h + 1) * 4, :],
                                      wv[:, h * 4:(h + 1) * 4, :])
                pmm = psum_mm.tile([B, DSL], f32, tag="pmm")
                for j in range(16):
                    nc.tensor.matmul(pmm, lhsT=feaT[:, j, :], rhs=wcT[:, j, :],
                                     start=(j == 0), stop=(j == 15))
                if c == 0:
                    nc.vector.tensor_scalar_mul(mapped, pmm, onehot[:, 0:1])
                else:
                    nc.vector.scalar_tensor_tensor(
                        out=mapped, in0=pmm, scalar=onehot[:, c:c + 1], in1=mapped,
                        op0=ALU.mult, op1=ALU.add)
            nc.vector.tensor_add(mapped, mapped, bwsl)

            # ---------- AllToAll: D-slices -> per-core full rows ----------
            ag_in = dram.tile([B, DSL], f32, tag="ag_in")
            ag_out = dram.tile([B, DSL], f32, tag="ag_out")
            nc.sync.dma_start(ag_in, mapped)
            if collective:
                nc.gpsimd.collective_compute(
                    "AllToAll", ALU.bypass,
                    replica_groups=[list(range(NCORES))],
                    ins=[ag_in[:, :].opt()], outs=[ag_out[:, :].opt()])
            else:
                # single-core timing variant: plain copy stands in for AllToAll
                nc.sync.dma_start(ag_out, ag_in[:, :])

            # ag_out[j*8 + p, :] = D-slice j of sample (8*core + p).
            # fold sample p's full row into [128, 8]: partition q holds D[8q..8q+8)
            mraw = state.tile([128, SAMP, FOLD], f32, tag="mraw")
            agv = ag_out[:, :].rearrange("(j p) (q c) -> j p q c", p=SAMP, c=FOLD)
            for p in range(SAMP):
                nc.sync.dma_start(mraw[:, p, :], agv[:, p, :, :])

            # ---------- loga = mraw - lse_D(mraw) per sample ----------
            def cross_partition_chain(src, nprob, op_alu):
                pr = scratch.tile([128, nprob], f32, tag="cp_pr")
                nc.vector.tensor_reduce(pr, src, axis=AX.X, op=op_alu)
                tp = psum_tr.tile([nprob, 128], f32, tag="tr")
                nc.tensor.transpose(tp, pr, ident)
                red = scratch.tile([nprob, 1], f32, tag="cp_red")
                nc.vector.tensor_reduce(red, tp, axis=AX.X, op=op_alu)
                return red

            def bcast_over_parts(col, nprob):
                tpc = psum_tr.tile([1, nprob], f32, tag="tr")
                nc.tensor.transpose(tpc, col, ident[0:nprob, 0:nprob])
                row = scratch.tile([1, nprob], f32, tag="bc_row")
                nc.scalar.copy(row, tpc)
                bc = psum_tr.tile([128, nprob], f32, tag="tr")
                nc.tensor.matmul(bc, lhsT=ones1, rhs=row, start=True, stop=True)
                return bc

            # mapped ~ N(0,1)-scale (|x| < ~10), so exp needs no max-stabilizer:
            # Za = ln(sum exp(mapped)) directly.
            e_e = scratch.tile([128, SAMP, FOLD], f32, tag="e_e")
            nc.scalar.activation(e_e, mraw, AF.Exp)
            ss_col = cross_partition_chain(e_e, SAMP, ALU.add)
            za = scratch.tile([SAMP, 1], f32, tag="za")
            nc.scalar.activation(za, ss_col, AF.Ln)
            bz = bcast_over_parts(za, SAMP)
            loga = state.tile([128, SAMP, FOLD], f32, tag="loga")
            nc.vector.tensor_sub(loga, mraw, bz[:, :, None].broadcast_to([128, SAMP, FOLD]))

            # ---------- marginals per group ----------
            # group A (9 probs): 0..7 ab (LA=loga_p, LB=logb_p), 8 bb (LA=LB=logb_cls)
            # group B (8 probs): aa (LA=LB=loga_p)
            NP = {0: NA, 1: NB}
            LA = {}; LB = {}
            LA[0] = state.tile([128, NA, FOLD], f32, tag="LA_A", name="LA_A")
            LB[0] = state.tile([128, NA, FOLD], f32, tag="LB_A", name="LB_A")
            LA[1] = state.tile([128, NB, FOLD], f32, tag="LA_B", name="LA_B")
            LB[1] = state.tile([128, NB, FOLD], f32, tag="LB_B", name="LB_B")
            nc.scalar.copy(LA[0][:, 0:SAMP, :], loga)
            nc.scalar.copy(LA[1][:, :, :], loga)
            nc.scalar.copy(LB[1][:, :, :], loga)
            for p in range(SAMP):
                nc.sync.dma_start(
                    LB[0][:, p, :],
                    logbs_d[p, :].rearrange("(q c) -> q c", c=FOLD))
            nc.sync.dma_start(
                LA[0][:, SAMP, :],
                logbs_d[SAMP, :].rearrange("(q c) -> q c", c=FOLD))
            nc.sync.dma_start(
                LB[0][:, SAMP, :],
                logbs_d[SAMP, :].rearrange("(q c) -> q c", c=FOLD))

            # ---------- Sinkhorn state (per group) ----------
            SFh = {}; SGh = {}; Mf = {}; Mg = {}; Tb = {}; Eb = {}
            acc = {}; lnacc = {}
            for g in (0, 1):
                s = "AB"[g]
                SFh[g] = state.tile([128, NP[g], HW], f32, tag=f"SFh{s}", name=f"SFh{s}")
                SGh[g] = state.tile([128, NP[g], HW], f32, tag=f"SGh{s}", name=f"SGh{s}")
                Mf[g] = state.tile([128, NP[g], FOLD], f32, tag=f"Mf{s}", name=f"Mf{s}")
                Mg[g] = state.tile([128, NP[g], FOLD], f32, tag=f"Mg{s}", name=f"Mg{s}")
                # band-slot order: [0]=d0, [1]=-1, [2]=+1, [3]=-2, [4]=+2, [5]=-3, [6]=+3
                Tb[g] = state.tile([128, NP[g], FOLD, 2 * R + 1], f32, tag=f"Tb{s}", name=f"Tb{s}")
                Eb[g] = state.tile([128, NP[g], FOLD, 2 * R + 1], f32, tag=f"Eb{s}", name=f"Eb{s}")
                acc[g] = state.tile([128, NP[g], FOLD], f32, tag=f"acc{s}", name=f"acc{s}")
                lnacc[g] = state.tile([128, NP[g], FOLD], f32, tag=f"lnacc{s}", name=f"lnacc{s}")
                nc.vector.memset(Mf[g], 0.0)
                nc.vector.memset(Mg[g], 0.0)
                nc.vector.memset(SFh[g], 0.0)
                nc.vector.memset(SGh[g], 0.0)

            def halo_fill(buf, g):
                np_ = NP[g]
                # both halos via one PSUM tile + one 2-range DVE add.
                # left halo: buf[q, :, 0:R] = buf[q-1, :, FOLD:FOLD+R]
                # right halo: buf[q, :, R+FOLD:HW] = buf[q+1, :, R:2R]
                # boundary partitions forced to NEG by the hmask tile.
                if HALO_DVE:
                    ph = psum_h.tile([128, 2, NA, R], f32, tag=f"halo{'AB'[g]}")
                    nc.tensor.matmul(ph[:, 0, 0:np_, :], lhsT=sdn,
                                     rhs=buf[:, :, FOLD:FOLD + R], start=True, stop=True)
                    nc.tensor.matmul(ph[:, 1, 0:np_, :], lhsT=sup,
                                     rhs=buf[:, :, R:2 * R], start=True, stop=True)
                    # one DVE add writes both halo col-ranges: dims (side, prob, col)
                    bb = buf[:, :, 0:R]
                    halo_out = bass.AP(tensor=bb.tensor, offset=bb.offset,
                                       ap=[bb.ap[0], [R + FOLD, 2], bb.ap[1], [1, R]])
                    ph_v = ph[:, :, 0:np_, :]
                    hs = hmask[:, :]
                    hm_v = bass.AP(tensor=hs.tensor, offset=hs.offset,
                                   ap=[hs.ap[0], [R, 2], [0, np_], [1, R]])
                    nc.vector.tensor_add(halo_out, ph_v, hm_v)
                else:
                    pl = psum_h.tile([128, NA, R], f32, tag=f"hl{'AB'[g]}")
                    nc.tensor.matmul(pl[:, 0:np_, :], lhsT=sdn,
                                     rhs=buf[:, :, FOLD:FOLD + R], start=True, stop=True)
                    nc.scalar.add(buf[:, :, 0:R], pl[:, 0:np_, :], bneg0[:, 0:1])
                    pr_ = psum_h.tile([128, NA, R], f32, tag=f"hr{'AB'[g]}")
                    nc.tensor.matmul(pr_[:, 0:np_, :], lhsT=sup,
                                     rhs=buf[:, :, R:2 * R], start=True, stop=True)
                    nc.scalar.add(buf[:, :, R + FOLD:HW], pr_[:, 0:np_, :],
                                  bneg127[:, 0:1])

            for g in (0, 1):
                nc.scalar.copy(SGh[g][:, :, R:R + FOLD], LB[g])
                nc.scalar.copy(SFh[g][:, :, R:R + FOLD], LA[g])
                halo_fill(SGh[g], g)

            def pair_ap(buf, k, np_):
                """[128, np, FOLD, 2] view of buf: pair (-k, +k) windows."""
                base = buf[:, :, R - k:R - k + FOLD]
                return bass.AP(tensor=base.tensor, offset=base.offset,
                               ap=[*base.ap, [2 * k, 2]])

            def bcast2_ap(t):
                """append a step-0 pair dim to a [128, np, FOLD] view."""
                return bass.AP(tensor=t.tensor, offset=t.offset,
                               ap=[*t.ap, [0, 2]])

            def half_update(g, src, dst, M, LAd, rmax=R):
                T, E = Tb[g], Eb[g]
                np_ = NP[g]
                # pair windows via 4D tensor_sub (walrus-legal); c_d goes into
                # the per-pair Exp bias (c identical for +-d). rmax limits the
                # band: |d|=3 terms are f32-invisible for aa/bb always and for
                # ab before iteration ~41 (validated end-to-end vs full f32).
                nc.vector.tensor_sub(T[:, :, :, 0], src[:, :, R:R + FOLD], M)
                for k in range(1, rmax + 1):
                    nc.vector.tensor_sub(
                        T[:, :, :, 2 * k - 1:2 * k + 1], pair_ap(src, k, np_),
                        bcast2_ap(M[:, :, :]))
                nc.scalar.activation(E[:, :, :, 0], T[:, :, :, 0], AF.Exp)
                for k in range(1, rmax + 1):
                    nc.scalar.activation(
                        E[:, :, :, 2 * k - 1:2 * k + 1], T[:, :, :, 2 * k - 1:2 * k + 1],
                        AF.Exp, bias=cbias[-20.0 * k * k][:, 0:1])
                nc.vector.tensor_reduce(acc[g], E[:, :, :, 0:2 * rmax + 1],
                                        axis=AX.X, op=ALU.add)
                nc.scalar.activation(lnacc[g], acc[g], AF.Ln)
                # dst center: LA - M_new == dst_prev - lnacc (dst was LA - M_old);
                # keeps the halo shift off the M-update dependency
                nc.vector.tensor_sub(dst[:, :, R:R + FOLD],
                                     dst[:, :, R:R + FOLD], lnacc[g])
                nc.gpsimd.tensor_add(M, M, lnacc[g])
                halo_fill(dst, g)

            TSW = 30
            for _t in range(iters):
                rA = 2 if _t < TSW else 3
                half_update(0, SGh[0], SFh[0], Mf[0], LA[0], rA)
                half_update(1, SGh[1], SFh[1], Mf[1], LA[1], 2)
                half_update(0, SFh[0], SGh[0], Mg[0], LB[0], rA)
                half_update(1, SFh[1], SGh[1], Mg[1], LB[1], 2)

            # ---------- value: V = sum_{i,d} exp(SF_i+SG_{i+d}+c_d)(F_i+G_{i+d}) ----------
            vrows = {}
            for g in (0, 1):
                np_ = NP[g]
                Ft = scratch.tile([128, NA, FOLD], f32, tag="Ft", name="Ft")[:, 0:np_, :]
                nc.vector.tensor_sub(Ft, SFh[g][:, :, R:R + FOLD], LA[g])
                nc.vector.tensor_scalar_mul(Ft, Ft, float(EPS))
                Gh = scratch.tile([128, NA, HW], f32, tag="Gh", name="Gh")[:, 0:np_, :]
                nc.vector.tensor_sub(Gh[:, :, R:R + FOLD], SGh[g][:, :, R:R + FOLD], LB[g])
                nc.vector.tensor_scalar_mul(
                    Gh[:, :, R:R + FOLD], Gh[:, :, R:R + FOLD], float(EPS))
                halo_fill(Gh, g)

                s1 = scratch.tile([128, NA, FOLD], f32, tag="s1", name="s1")[:, 0:np_, :]
                pd_ = scratch.tile([128, NA, FOLD], f32, tag="pd", name="pd")[:, 0:np_, :]
                s2 = scratch.tile([128, NA, FOLD], f32, tag="s2", name="s2")[:, 0:np_, :]
                vacc = scratch.tile([128, NA, FOLD], f32, tag="vacc", name="vacc")[:, 0:np_, :]
                for di, d in enumerate(range(-R, R + 1)):
                    nc.vector.tensor_add(
                        s1, SFh[g][:, :, R:R + FOLD], SGh[g][:, :, R + d:R + d + FOLD])
                    nc.scalar.activation(pd_, s1, AF.Exp,
                                         bias=cbias[-20.0 * d * d][:, 0:1])
                    nc.vector.tensor_add(s2, Ft, Gh[:, :, R + d:R + d + FOLD])
                    if di == 0:
                        nc.vector.tensor_mul(vacc, pd_, s2)
                    else:
                        nc.vector.tensor_mul(s2, pd_, s2)
                        nc.vector.tensor_add(vacc, vacc, s2)

                vsum = scratch.tile([128, NA], f32, tag="vsum", name="vsum")[:, 0:np_]
                nc.vector.tensor_reduce(vsum, vacc, axis=AX.X, op=ALU.add)
                tv = psum_tr.tile([NA, 128], f32, tag="tr")
                nc.tensor.transpose(tv[0:np_, :], vsum, ident)
                vrow = scratch.tile([NA, 1], f32, tag=f"vrow{'AB'[g]}", name=f"vrow{'AB'[g]}")
                nc.vector.tensor_reduce(vrow[0:np_, :], tv[0:np_, :], axis=AX.X,
                                        op=ALU.add)
                vrows[g] = vrow

            # output layout: [0:8] ab, [8] bb, [9:17] aa
            nc.sync.dma_start(vals_d[0:NA], vrows[0][0:NA, :])
            nc.sync.dma_start(vals_d[NA:NPROB], vrows[1][0:NB, :])

    if not nc.is_finalized():
        # All ScalarE funcs used here (Exp, Ln, Identity, Copy) live in the
        # single act-table set "natural_log_exp_and_others"; hide the other
        # sets (preserving list positions = act_func_set_id) so the table
        # load pass emits one load instead of thrashing between
        # exp_and_others and natural_log every half-update.
        import concourse.bacc as bacc_mod
        _orig_gat = bacc_mod.get_activation_tables
        def _one_set_gat(arch):
            t = _orig_gat(arch)
            return {name: (fns if name == "natural_log_exp_and_others" else set())
                    for name, fns in t.items()}
        bacc_mod.get_activation_tables = _one_set_gat
        try:
            nc.finalize()
        finally:
            bacc_mod.get_activation_tables = _orig_gat
    return nc


def _log_softmax(x):
    m = x.max(axis=-1, keepdims=True)
    e = np.exp(x - m)
    return (x - m) - np.log(e.sum(axis=-1, keepdims=True))


def prep_in_maps(fea, y, W, bW, anchor_tab):
    fea = np.ascontiguousarray(np.asarray(fea, dtype=np.float32))
    y = np.asarray(y).astype(np.int64)
    W = np.asarray(W, dtype=np.float32)
    bW = np.asarray(bW, dtype=np.float32)
    anchor_tab = np.asarray(anchor_tab, dtype=np.float32)

    logb_cls = _log_softmax(anchor_tab).astype(np.float32)      # [5, D]
    bW_g = bW[y]                                                # [B, D]
    onehot = np.zeros((B, NUM_CLASSES), dtype=np.float32)
    onehot[np.arange(B), y] = 1.0

    in_maps = []
    for k in range(NCORES):
        sl = slice(k * DSL, (k + 1) * DSL)
        logbs = np.empty((SAMP + 1, D), dtype=np.float32)
        for p in range(SAMP):
            logbs[p] = logb_cls[y[k * SAMP + p]]
        logbs[SAMP] = logb_cls[k] if k < NUM_CLASSES else logb_cls[0]
        in_maps.append({
            "fea": fea,
            "wsl": np.ascontiguousarray(W[:, sl, :].transpose(0, 2, 1)),
            "bwsl": np.ascontiguousarray(bW_g[:, sl]),
            "onehot": onehot,
            "logbs": logbs,
        })
    return in_maps


def postprocess(vals_per_core, y):
    y = np.asarray(y).astype(np.int64)
    V_ab = np.empty(B, dtype=np.float32)
    V_aa = np.empty(B, dtype=np.float32)
    V_bb = np.empty(NUM_CLASSES, dtype=np.float32)
    for k in range(NCORES):
        v = np.asarray(vals_per_core[k]).reshape(-1)
        V_ab[k * SAMP:(k + 1) * SAMP] = v[0:SAMP]
        V_aa[k * SAMP:(k + 1) * SAMP] = v[NA:NPROB]
        if k < NUM_CLASSES:
            V_bb[k] = v[SAMP]
    losses = (2.0 * V_ab - V_aa - V_bb[y]).astype(np.float32)
    z = -losses * np.float32(TEMP)
    zm = z.max()
    e = np.exp(z - zm)
    weights = (e / e.sum()).astype(np.float32)
    loss = np.float32(np.sum(losses * weights))
    return loss, weights


_CACHE = {}


def kernel(fea, y, W, bW, anchor_tab):
    from concourse.bass_utils import run_bass_kernel_spmd

    if "nc" not in _CACHE:
        _CACHE["nc"] = _build_program(ITERS)
    nc = _CACHE["nc"]

    in_maps = prep_in_maps(fea, y, W, bW, anchor_tab)
    res = run_bass_kernel_spmd(nc, in_maps, list(range(NCORES)))
    vals = [res.results[k]["values"] for k in range(NCORES)]
    return postprocess(vals, y)


# revision 39
# speedup vs baseline: 1.1887x; 1.0365x over previous
"""Trainium2 Bass kernel for the Sinkhorn-OT loss problem.

Math summary (mirrors the reference):
  mapped = einsum('bf,bdf->bd', fea, W[y]) + bW[y];  a = softmax(mapped)
  b = softmax(anchor_tab[y]);  M_ij = (i-j)^2 on D=1024, eps=0.05
  ot(x, y) = 50-iteration log-domain Sinkhorn value
  losses = 2*ot(a,b) - ot(a,a) - ot(b,b);  weights = softmax(-losses)
  loss = sum(losses*weights)

Key structural facts exploited:
  * The Gibbs kernel exp(-(i-j)^2/eps) decays by e^-20 per unit distance, so
    every logsumexp over D is exactly banded with radius 3 in f32 (terms at
    |d|=4 sit >= 40 below the row max - invisible at f32 precision).
  * ot(b,b) has only 5 distinct instances (one per class).
  * The warm-start stabilizer (previous iteration's lse output) keeps all
    exp arguments in [-inf, ~7]; no per-row max pass is needed.

Distribution over the 8 cores:
  Phase A: D-parallel matmul. Core k holds W[:, 128k:128k+128, :], computes
    that D-slice of mapped for all 64 samples on the TensorEngine, then an
    AllToAll redistributes slices so core k owns full rows for samples
    8k..8k+8.
  Phase B: problem-parallel Sinkhorn. Each core runs 17 independent solves
    in two interleaved groups (A: 8 ab + 1 bb, B: 8 aa) so the two serial
    update chains fill each other's pipeline gaps. Each problem's D=1024
    axis is folded as [128 partitions x 8 cols] plus a 3-wide halo
    maintained by TensorEngine shift-matmuls.
  Host: assembles the 133 values, forms losses/weights/loss (O(64) work).

Problem order in the "values" output: [0:8] ab, [8] bb, [9:17] aa.
"""

import numpy as np

NUM_CLASSES = 5
FEAT = 2048
D = 1024
B = 64
EPS = 0.05
ITERS = 50
TEMP = 1.0

R = 3                 # band radius
HW = 8 + 2 * R        # halo'd fold width (14)
NCORES = 8
SAMP = B // NCORES    # samples per core (8)
NPROB = 2 * SAMP + 1  # problems per core (17)
DSL = D // NCORES     # D-slice per core (128)
FOLD = D // 128       # fold width (8)
NEG = -1.0e30
import os
EXP_SPLIT = int(os.environ.get('EXP_SPLIT', '1'))
HALO_DVE = int(os.environ.get('HALO_DVE', '0'))
NA = SAMP + 1         # group A: 8 ab + 1 bb
NB = SAMP             # group B: 8 aa


def _build_program(iters=ITERS, collective=True):
    import concourse.bass as bass
    import concourse.bacc as bacc
    import concourse.tile as tile
    from concourse import mybir

    f32 = mybir.dt.float32
    AF = mybir.ActivationFunctionType
    ALU = mybir.AluOpType
    AX = mybir.AxisListType

    nc = bacc.Bacc("TRN2", target_bir_lowering=False, debug=False)

    fea_d = nc.declare_dram_parameter("fea", [B, FEAT], f32, isOutput=False)
    wsl_d = nc.declare_dram_parameter("wsl", [NUM_CLASSES, FEAT, DSL], f32, isOutput=False)
    bwsl_d = nc.declare_dram_parameter("bwsl", [B, DSL], f32, isOutput=False)
    onehot_d = nc.declare_dram_parameter("onehot", [B, NUM_CLASSES], f32, isOutput=False)
    logbs_d = nc.declare_dram_parameter("logbs", [SAMP + 1, D], f32, isOutput=False)
    vals_d = nc.declare_dram_parameter("values", [NPROB], f32, isOutput=True)

    ident_d = nc.inline_tensor(np.eye(128, dtype=np.float32), name="ident")
    # shift-down: out[q] = in[q-1]  (SD[k, m] = 1 iff k = m-1)
    sd_d = nc.inline_tensor(np.eye(128, k=1, dtype=np.float32), name="sdn")
    # shift-up: out[q] = in[q+1]  (SU[k, m] = 1 iff k = m+1)
    su_d = nc.inline_tensor(np.eye(128, k=-1, dtype=np.float32), name="sup")
    ones_d = nc.inline_tensor(np.ones((1, 128), dtype=np.float32), name="ones1")
    bneg0_np = np.zeros((128, 1), dtype=np.float32); bneg0_np[0, 0] = NEG
    bneg127_np = np.zeros((128, 1), dtype=np.float32); bneg127_np[127, 0] = NEG
    bneg0_d = nc.inline_tensor(bneg0_np, name="bneg0")
    bneg127_d = nc.inline_tensor(bneg127_np, name="bneg127")
    # halo boundary mask: cols 0:3 NEG at partition 0, cols 3:6 NEG at partition 127
    hm_np = np.zeros((128, 6), dtype=np.float32)
    hm_np[0, 0:3] = NEG; hm_np[127, 3:6] = NEG
    hmask_d = nc.inline_tensor(hm_np, name="hmask")

    with tile.TileContext(nc) as tc:
        with (
            tc.tile_pool(name="consts", bufs=1) as consts,
            tc.tile_pool(name="pha", bufs=2) as pha,
            tc.tile_pool(name="state", bufs=1) as state,
            tc.tile_pool(name="scratch", bufs=2) as scratch,
            tc.tile_pool(name="psum_mm", bufs=2, space="PSUM") as psum_mm,
            tc.tile_pool(name="psum_tr", bufs=2, space="PSUM") as psum_tr,
            tc.tile_pool(name="psum_h", bufs=1, space="PSUM") as psum_h,
            tc.tile_pool(name="dram", bufs=1, space="DRAM") as dram,
        ):
            # ---------- constants to SBUF ----------
            ident = consts.tile([128, 128], f32, tag="ident")
            nc.sync.dma_start(ident, ident_d[:, :])
            sdn = consts.tile([128, 128], f32, tag="sdn")
            nc.sync.dma_start(sdn, sd_d[:, :])
            sup = consts.tile([128, 128], f32, tag="sup")
            nc.sync.dma_start(sup, su_d[:, :])
            ones1 = consts.tile([1, 128], f32, tag="ones1")
            nc.sync.dma_start(ones1, ones_d[:, :])
            onehot = consts.tile([B, NUM_CLASSES], f32, tag="onehot")
            nc.sync.dma_start(onehot, onehot_d[:, :])
            bwsl = consts.tile([B, DSL], f32, tag="bwsl")
            nc.sync.dma_start(bwsl, bwsl_d[:, :])
            bneg0 = consts.tile([128, 1], f32, tag="bneg0")
            nc.sync.dma_start(bneg0, bneg0_d[:, :])
            bneg127 = consts.tile([128, 1], f32, tag="bneg127")
            nc.sync.dma_start(bneg127, bneg127_d[:, :])
            hmask = consts.tile([128, 6], f32, tag="hmask")
            nc.sync.dma_start(hmask, hmask_d[:, :])

            # per-partition bias constants for activation (c_d = -20*d^2)
            cbias = {}
            for v in sorted({-20.0 * d * d for d in range(0, R + 1)}):
                t = consts.tile([128, 1], f32, tag=f"cb{int(-v)}")
                nc.vector.memset(t, v)
                cbias[v] = t

            # ---------- Phase A: mapped D-slice for all 64 samples ----------
            fea_sb = consts.tile([B, FEAT], f32, tag="fea")
            nc.sync.dma_start(fea_sb, fea_d[:, :])

            feaT = consts.tile([128, 16, B], f32, tag="feaT")
            for j in range(16):
                pt = psum_tr.tile([128, B], f32, tag="tr")
                nc.tensor.transpose(pt, fea_sb[:, j * 128:(j + 1) * 128], ident[0:B, 0:B])
                if j % 2 == 0:
                    nc.vector.tensor_copy(feaT[:, j, :], pt)
                else:
                    nc.scalar.copy(feaT[:, j, :], pt)

            mapped = state.tile([B, DSL], f32, tag="mapped")
            for c in range(NUM_CLASSES):
                # W arrives host-pre-transposed as [C, FEAT, DSL]: DMA the
                # [feat-chunk, d] tiles straight into matmul-ready layout.
                wcT = pha.tile([128, 16, DSL], f32, tag="wcT")
                wv = wsl_d[c, :, :].rearrange("(j p) d -> p j d", p=128)
                for h in range(4):
                    nc.sync.dma_start(wcT[:, h * 4:(h + 1) * 4, :],
                                      wv[:, h * 4:(h + 1) * 4, :])
                pmm = psum_mm.tile([B, DSL], f32, tag="pmm")
                for j in range(16):
                    nc.tensor.matmul(pmm, lhsT=feaT[:, j, :], rhs=wcT[:, j, :],
                                     start=(j == 0), stop=(j == 15))
                if c == 0:
                    nc.vector.tensor_scalar_mul(mapped, pmm, onehot[:, 0:1])
                else:
                    nc.vector.scalar_tensor_tensor(
                        out=mapped, in0=pmm, scalar=onehot[:, c:c + 1], in1=mapped,
                        op0=ALU.mult, op1=ALU.add)
            nc.vector.tensor_add(mapped, mapped, bwsl)

            # ---------- AllToAll: D-slices -> per-core full rows ----------
            ag_in = dram.tile([B, DSL], f32, tag="ag_in")
            ag_out = dram.tile([B, DSL], f32, tag="ag_out")
            nc.sync.dma_start(ag_in, mapped)
            if collective:
                nc.gpsimd.collective_compute(
                    "AllToAll", ALU.bypass,
                    replica_groups=[list(range(NCORES))],
                    ins=[ag_in[:, :].opt()], outs=[ag_out[:, :].opt()])
            else:
                # single-core timing variant: plain copy stands in for AllToAll
                nc.sync.dma_start(ag_out, ag_in[:, :])

            # ag_out[j*8 + p, :] = D-slice j of sample (8*core + p).
            # fold sample p's full row into [128, 8]: partition q holds D[8q..8q+8)
            mraw = state.tile([128, SAMP, FOLD], f32, tag="mraw")
            agv = ag_out[:, :].rearrange("(j p) (q c) -> j p q c", p=SAMP, c=FOLD)
            for p in range(SAMP):
                nc.sync.dma_start(mraw[:, p, :], agv[:, p, :, :])

            # ---------- loga = mraw - lse_D(mraw) per sample ----------
            def cross_partition_chain(src, nprob, op_alu):
                pr = scratch.tile([128, nprob], f32, tag="cp_pr")
                nc.vector.tensor_reduce(pr, src, axis=AX.X, op=op_alu)
                tp = psum_tr.tile([nprob, 128], f32, tag="tr")
                nc.tensor.transpose(tp, pr, ident)
                red = scratch.tile([nprob, 1], f32, tag="cp_red")
                nc.vector.tensor_reduce(red, tp, axis=AX.X, op=op_alu)
                return red

            def bcast_over_parts(col, nprob):
                tpc = psum_tr.tile([1, nprob], f32, tag="tr")
                nc.tensor.transpose(tpc, col, ident[0:nprob, 0:nprob])
                row = scratch.tile([1, nprob], f32, tag="bc_row")
                nc.scalar.copy(row, tpc)
                bc = psum_tr.tile([128, nprob], f32, tag="tr")
                nc.tensor.matmul(bc, lhsT=ones1, rhs=row, start=True, stop=True)
                return bc

            # mapped ~ N(0,1)-scale (|x| < ~10), so exp needs no max-stabilizer:
            # Za = ln(sum exp(mapped)) directly.
            e_e = scratch.tile([128, SAMP, FOLD], f32, tag="e_e")
            nc.scalar.activation(e_e, mraw, AF.Exp)
            ss_col = cross_partition_chain(e_e, SAMP, ALU.add)
            za = scratch.tile([SAMP, 1], f32, tag="za")
            nc.scalar.activation(za, ss_col, AF.Ln)
            bz = bcast_over_parts(za, SAMP)
            loga = state.tile([128, SAMP, FOLD], f32, tag="loga")
            nc.vector.tensor_sub(loga, mraw, bz[:, :, None].broadcast_to([128, SAMP, FOLD]))

            # ---------- marginals per group ----------
            # group A (9 probs): 0..7 ab (LA=loga_p, LB=logb_p), 8 bb (LA=LB=logb_cls)
            # group B (8 probs): aa (LA=LB=loga_p)
            NP = {0: NA, 1: NB}
            LA = {}; LB = {}
            LA[0] = state.tile([128, NA, FOLD], f32, tag="LA_A", name="LA_A")
            LB[0] = state.tile([128, NA, FOLD], f32, tag="LB_A", name="LB_A")
            LA[1] = state.tile([128, NB, FOLD], f32, tag="LA_B", name="LA_B")
            LB[1] = state.tile([128, NB, FOLD], f32, tag="LB_B", name="LB_B")
            nc.scalar.copy(LA[0][:, 0:SAMP, :], loga)
            nc.scalar.copy(LA[1][:, :, :], loga)
            nc.scalar.copy(LB[1][:, :, :], loga)
            for p in range(SAMP):
                nc.sync.dma_start(
                    LB[0][:, p, :],
                    logbs_d[p, :].rearrange("(q c) -> q c", c=FOLD))
            nc.sync.dma_start(
                LA[0][:, SAMP, :],
                logbs_d[SAMP, :].rearrange("(q c) -> q c", c=FOLD))
            nc.sync.dma_start(
                LB[0][:, SAMP, :],
                logbs_d[SAMP, :].rearrange("(q c) -> q c", c=FOLD))

            # ---------- Sinkhorn state (per group) ----------
            SFh = {}; SGh = {}; Mf = {}; Mg = {}; Tb = {}; Eb = {}
            acc = {}; lnacc = {}
            for g in (0, 1):
                s = "AB"[g]
                SFh[g] = state.tile([128, NP[g], HW], f32, tag=f"SFh{s}", name=f"SFh{s}")
                SGh[g] = state.tile([128, NP[g], HW], f32, tag=f"SGh{s}", name=f"SGh{s}")
                Mf[g] = state.tile([128, NP[g], FOLD], f32, tag=f"Mf{s}", name=f"Mf{s}")
                Mg[g] = state.tile([128, NP[g], FOLD], f32, tag=f"Mg{s}", name=f"Mg{s}")
                # band-slot order: [0]=d0, [1]=-1, [2]=+1, [3]=-2, [4]=+2, [5]=-3, [6]=+3
                Tb[g] = state.tile([128, NP[g], FOLD, 2 * R + 1], f32, tag=f"Tb{s}", name=f"Tb{s}")
                Eb[g] = state.tile([128, NP[g], FOLD, 2 * R + 1], f32, tag=f"Eb{s}", name=f"Eb{s}")
                acc[g] = state.tile([128, NP[g], FOLD], f32, tag=f"acc{s}", name=f"acc{s}")
                lnacc[g] = state.tile([128, NP[g], FOLD], f32, tag=f"lnacc{s}", name=f"lnacc{s}")
                nc.vector.memset(Mf[g], 0.0)
                nc.vector.memset(Mg[g], 0.0)
                nc.vector.memset(SFh[g], 0.0)
                nc.vector.memset(SGh[g], 0.0)

            def halo_fill(buf, g):
                np_ = NP[g]
                # both halos via one PSUM tile + one 2-range DVE add.
                # left halo: buf[q, :, 0:R] = buf[q-1, :, FOLD:FOLD+R]
                # right halo: buf[q, :, R+FOLD:HW] = buf[q+1, :, R:2R]
                # boundary partitions forced to NEG by the hmask tile.
                if HALO_DVE:
                    ph = psum_h.tile([128, 2, NA, R], f32, tag=f"halo{'AB'[g]}")
                    nc.tensor.matmul(ph[:, 0, 0:np_, :], lhsT=sdn,
                                     rhs=buf[:, :, FOLD:FOLD + R], start=True, stop=True)
                    nc.tensor.matmul(ph[:, 1, 0:np_, :], lhsT=sup,
                                     rhs=buf[:, :, R:2 * R], start=True, stop=True)
                    # one DVE add writes both halo col-ranges: dims (side, prob, col)
                    bb = buf[:, :, 0:R]
                    halo_out = bass.AP(tensor=bb.tensor, offset=bb.offset,
                                       ap=[bb.ap[0], [R + FOLD, 2], bb.ap[1], [1, R]])
                    ph_v = ph[:, :, 0:np_, :]
                    hs = hmask[:, :]
                    hm_v = bass.AP(tensor=hs.tensor, offset=hs.offset,
                                   ap=[hs.ap[0], [R, 2], [0, np_], [1, R]])
                    nc.vector.tensor_add(halo_out, ph_v, hm_v)
                else:
                    pl = psum_h.tile([128, NA, R], f32, tag=f"hl{'AB'[g]}")
                    nc.tensor.matmul(pl[:, 0:np_, :], lhsT=sdn,
                                     rhs=buf[:, :, FOLD:FOLD + R], start=True, stop=True)
                    nc.scalar.add(buf[:, :, 0:R], pl[:, 0:np_, :], bneg0[:, 0:1])
                    pr_ = psum_h.tile([128, NA, R], f32, tag=f"hr{'AB'[g]}")
                    nc.tensor.matmul(pr_[:, 0:np_, :], lhsT=sup,
                                     rhs=buf[:, :, R:2 * R], start=True, stop=True)
                    nc.scalar.add(buf[:, :, R + FOLD:HW], pr_[:, 0:np_, :],
                                  bneg127[:, 0:1])

            for g in (0, 1):
                nc.scalar.copy(SGh[g][:, :, R:R + FOLD], LB[g])
                nc.scalar.copy(SFh[g][:, :, R:R + FOLD], LA[g])
                halo_fill(SGh[g], g)

            def pair_ap(buf, k, np_):
                """[128, np, FOLD, 2] view of buf: pair (-k, +k) windows."""
                base = buf[:, :, R - k:R - k + FOLD]
                return bass.AP(tensor=base.tensor, offset=base.offset,
                               ap=[*base.ap, [2 * k, 2]])

            def bcast2_ap(t):
                """append a step-0 pair dim to a [128, np, FOLD] view."""
                return bass.AP(tensor=t.tensor, offset=t.offset,
                               ap=[*t.ap, [0, 2]])

            def half_update(g, src, dst, M, LAd, rmax=R):
                T, E = Tb[g], Eb[g]
                np_ = NP[g]
                # pair windows via 4D tensor_sub (walrus-legal); c_d goes into
                # the per-pair Exp bias (c identical for +-d). rmax limits the
                # band: |d|=3 terms are f32-invisible for aa/bb always and for
                # ab before iteration ~41 (validated end-to-end vs full f32).
                nc.vector.tensor_sub(T[:, :, :, 0], src[:, :, R:R + FOLD], M)
                for k in range(1, rmax + 1):
                    nc.vector.tensor_sub(
                        T[:, :, :, 2 * k - 1:2 * k + 1], pair_ap(src, k, np_),
                        bcast2_ap(M[:, :, :]))
                nc.scalar.activation(E[:, :, :, 0], T[:, :, :, 0], AF.Exp)
                for k in range(1, rmax + 1):
                    nc.scalar.activation(
                        E[:, :, :, 2 * k - 1:2 * k + 1], T[:, :, :, 2 * k - 1:2 * k + 1],
                        AF.Exp, bias=cbias[-20.0 * k * k][:, 0:1])
                nc.vector.tensor_reduce(acc[g], E[:, :, :, 0:2 * rmax + 1],
                                        axis=AX.X, op=ALU.add)
                nc.scalar.activation(lnacc[g], acc[g], AF.Ln)
                # dst center: LA - M_new == dst_prev - lnacc (dst was LA - M_old);
                # keeps the halo shift off the M-update dependency
                nc.vector.tensor_sub(dst[:, :, R:R + FOLD],
                                     dst[:, :, R:R + FOLD], lnacc[g])
                nc.gpsimd.tensor_add(M, M, lnacc[g])
                halo_fill(dst, g)

            # band-radius schedule (validated end-to-end vs full f32):
            # ab needs |d|=2 from t~10 and |d|=3 from t~41 (potential drift
            # grows with t); symmetric aa/bb problems never need |d|>=2.
            for _t in range(iters):
                rA = 1 if _t < 6 else (2 if _t < 30 else 3)
                half_update(0, SGh[0], SFh[0], Mf[0], LA[0], rA)
                half_update(1, SGh[1], SFh[1], Mf[1], LA[1], 1)
                half_update(0, SFh[0], SGh[0], Mg[0], LB[0], rA)
                half_update(1, SFh[1], SGh[1], Mg[1], LB[1], 1)

            # ---------- value: V = sum_{i,d} exp(SF_i+SG_{i+d}+c_d)(F_i+G_{i+d}) ----------
            vrows = {}
            for g in (0, 1):
                np_ = NP[g]
                Ft = scratch.tile([128, NA, FOLD], f32, tag="Ft", name="Ft")[:, 0:np_, :]
                nc.vector.tensor_sub(Ft, SFh[g][:, :, R:R + FOLD], LA[g])
                nc.vector.tensor_scalar_mul(Ft, Ft, float(EPS))
                Gh = scratch.tile([128, NA, HW], f32, tag="Gh", name="Gh")[:, 0:np_, :]
                nc.vector.tensor_sub(Gh[:, :, R:R + FOLD], SGh[g][:, :, R:R + FOLD], LB[g])
                nc.vector.tensor_scalar_mul(
                    Gh[:, :, R:R + FOLD], Gh[:, :, R:R + FOLD], float(EPS))
                halo_fill(Gh, g)

                s1 = scratch.tile([128, NA, FOLD], f32, tag="s1", name="s1")[:, 0:np_, :]
                pd_ = scratch.tile([128, NA, FOLD], f32, tag="pd", name="pd")[:, 0:np_, :]
                s2 = scratch.tile([128, NA, FOLD], f32, tag="s2", name="s2")[:, 0:np_, :]
                vacc = scratch.tile([128, NA, FOLD], f32, tag="vacc", name="vacc")[:, 0:np_, :]
                for di, d in enumerate(range(-R, R + 1)):
                    nc.vector.tensor_add(
                        s1, SFh[g][:, :, R:R + FOLD], SGh[g][:, :, R + d:R + d + FOLD])
                    nc.scalar.activation(pd_, s1, AF.Exp,
                                         bias=cbias[-20.0 * d * d][:, 0:1])
                    nc.vector.tensor_add(s2, Ft, Gh[:, :, R + d:R + d + FOLD])
                    if di == 0:
                        nc.vector.tensor_mul(vacc, pd_, s2)
                    else:
                        nc.vector.tensor_mul(s2, pd_, s2)
                        nc.vector.tensor_add(vacc, vacc, s2)

                vsum = scratch.tile([128, NA], f32, tag="vsum", name="vsum")[:, 0:np_]
                nc.vector.tensor_reduce(vsum, vacc, axis=AX.X, op=ALU.add)
                tv = psum_tr.tile([NA, 128], f32, tag="tr")
                nc.tensor.transpose(tv[0:np_, :], vsum, ident)
                vrow = scratch.tile([NA, 1], f32, tag=f"vrow{'AB'[g]}", name=f"vrow{'AB'[g]}")
                nc.vector.tensor_reduce(vrow[0:np_, :], tv[0:np_, :], axis=AX.X,
                                        op=ALU.add)
                vrows[g] = vrow

            # output layout: [0:8] ab, [8] bb, [9:17] aa
            nc.sync.dma_start(vals_d[0:NA], vrows[0][0:NA, :])
            nc.sync.dma_start(vals_d[NA:NPROB], vrows[1][0:NB, :])

    if not nc.is_finalized():
        # All ScalarE funcs used here (Exp, Ln, Identity, Copy) live in the
        # single act-table set "natural_log_exp_and_others"; hide the other
        # sets (preserving list positions = act_func_set_id) so the table
        # load pass emits one load instead of thrashing between
        # exp_and_others and natural_log every half-update.
        import concourse.bacc as bacc_mod
        _orig_gat = bacc_mod.get_activation_tables
        def _one_set_gat(arch):
            t = _orig_gat(arch)
            return {name: (fns if name == "natural_log_exp_and_others" else set())
                    for name, fns in t.items()}
        bacc_mod.get_activation_tables = _one_set_gat
        try:
            nc.finalize()
        finally:
            bacc_mod.get_activation_tables = _orig_gat
    return nc


def _log_softmax(x):
    m = x.max(axis=-1, keepdims=True)
    e = np.exp(x - m)
    return (x - m) - np.log(e.sum(axis=-1, keepdims=True))


def prep_in_maps(fea, y, W, bW, anchor_tab):
    fea = np.ascontiguousarray(np.asarray(fea, dtype=np.float32))
    y = np.asarray(y).astype(np.int64)
    W = np.asarray(W, dtype=np.float32)
    bW = np.asarray(bW, dtype=np.float32)
    anchor_tab = np.asarray(anchor_tab, dtype=np.float32)

    logb_cls = _log_softmax(anchor_tab).astype(np.float32)      # [5, D]
    bW_g = bW[y]                                                # [B, D]
    onehot = np.zeros((B, NUM_CLASSES), dtype=np.float32)
    onehot[np.arange(B), y] = 1.0

    in_maps = []
    for k in range(NCORES):
        sl = slice(k * DSL, (k + 1) * DSL)
        logbs = np.empty((SAMP + 1, D), dtype=np.float32)
        for p in range(SAMP):
            logbs[p] = logb_cls[y[k * SAMP + p]]
        logbs[SAMP] = logb_cls[k] if k < NUM_CLASSES else logb_cls[0]
        in_maps.append({
            "fea": fea,
            "wsl": np.ascontiguousarray(W[:, sl, :].transpose(0, 2, 1)),
            "bwsl": np.ascontiguousarray(bW_g[:, sl]),
            "onehot": onehot,
            "logbs": logbs,
        })
    return in_maps


def postprocess(vals_per_core, y):
    y = np.asarray(y).astype(np.int64)
    V_ab = np.empty(B, dtype=np.float32)
    V_aa = np.empty(B, dtype=np.float32)
    V_bb = np.empty(NUM_CLASSES, dtype=np.float32)
    for k in range(NCORES):
        v = np.asarray(vals_per_core[k]).reshape(-1)
        V_ab[k * SAMP:(k + 1) * SAMP] = v[0:SAMP]
        V_aa[k * SAMP:(k + 1) * SAMP] = v[NA:NPROB]
        if k < NUM_CLASSES:
            V_bb[k] = v[SAMP]
    losses = (2.0 * V_ab - V_aa - V_bb[y]).astype(np.float32)
    z = -losses * np.float32(TEMP)
    zm = z.max()
    e = np.exp(z - zm)
    weights = (e / e.sum()).astype(np.float32)
    loss = np.float32(np.sum(losses * weights))
    return loss, weights


_CACHE = {}


def kernel(fea, y, W, bW, anchor_tab):
    from concourse.bass_utils import run_bass_kernel_spmd

    if "nc" not in _CACHE:
        _CACHE["nc"] = _build_program(ITERS)
    nc = _CACHE["nc"]

    in_maps = prep_in_maps(fea, y, W, bW, anchor_tab)
    res = run_bass_kernel_spmd(nc, in_maps, list(range(NCORES)))
    vals = [res.results[k]["values"] for k in range(NCORES)]
    return postprocess(vals, y)
